# revision 1
# baseline (speedup 1.0000x reference)
import os
import sys

for _p in ("/opt/trn_rl_repo", "/root/.axon_site/_ro/trn_rl_repo"):
    if os.path.isdir(_p) and _p not in sys.path:
        sys.path.insert(0, _p)

import numpy as np
import ml_dtypes

import concourse.bass as bass
import concourse.tile as tile
import concourse.mybir as mybir
from concourse import bacc
from concourse._compat import axon_active
from concourse.bass import ts
from concourse.bass_utils import run_bass_kernel_spmd
from concourse.masks import make_identity

N_CORES = 8
D = 1024
F = 2048
T = 1024  # tokens per core (8192 / 8)

BF16 = mybir.dt.bfloat16
F32 = mybir.dt.float32


def build(nc, T=T, D=D, F=F, use_silu=True, psg_b=3, psu_b=3, psy_b=2,
          w_b=4, hb_extra=6, xf_b=3):
    """Emit the per-core MoE FFN kernel.

    Layout: activations transposed (feature on partitions, tokens on free dim).
    Paths: [shared, expert0, expert1]; expert token masks folded into the
    input (x0 = x*m0, x1 = x - x0) so all three paths sum directly.
    """
    KD = D // 128   # k-tiles over D (gate/up contraction, also out tiles of down)
    MF = F // 128   # m-tiles over F
    MD = D // 128
    KF = F // 128
    TH = T // 512   # 512-token free-dim blocks

    xt32 = nc.dram_tensor("xt32", [D, T], F32, kind="ExternalInput").ap()
    xtb = nc.dram_tensor("xtb", [D, T], BF16, kind="ExternalInput").ap()
    wr = nc.dram_tensor("wr", [128, KD, 2], F32, kind="ExternalInput").ap()
    rb = nc.dram_tensor("rb", [1, 2], F32, kind="ExternalInput").ap()
    wgl = nc.dram_tensor("wgl", [3 * MF, 128, KD, 128], BF16, kind="ExternalInput").ap()
    wul = nc.dram_tensor("wul", [3 * MF, 128, KD, 128], BF16, kind="ExternalInput").ap()
    wdl = nc.dram_tensor("wdl", [3 * MD, 128, KF, 128], BF16, kind="ExternalInput").ap()
    yt = nc.dram_tensor("yt", [D, T], F32, kind="ExternalOutput").ap()

    with tile.TileContext(nc) as tc:
        with (
            tc.tile_pool(name="xres", bufs=1) as xres,
            tc.tile_pool(name="xf", bufs=xf_b) as xf,
            tc.tile_pool(name="small", bufs=1) as small,
            tc.tile_pool(name="wg", bufs=w_b) as wgp,
            tc.tile_pool(name="wu", bufs=w_b) as wup,
            tc.tile_pool(name="wd", bufs=w_b) as wdp,
            tc.tile_pool(name="hb", bufs=KF + hb_extra) as hb,
            tc.tile_pool(name="gs", bufs=3) as gsp,
            tc.tile_pool(name="yac", bufs=1) as yac,
            tc.tile_pool(name="psg", bufs=psg_b, space="PSUM") as psg,
            tc.tile_pool(name="psu", bufs=psu_b, space="PSUM") as psu,
            tc.tile_pool(name="psy", bufs=psy_b, space="PSUM") as psy,
        ):
            # resident transposed input (bf16) + masked variants
            xtb_sb = xres.tile([128, KD, T], BF16, tag="xtb")
            xtb_r = xtb.rearrange("(ko p) t -> p ko t", p=128)
            for ko in range(KD):
                nc.sync.dma_start(xtb_sb[:, ko, :], xtb_r[:, ko, :])
            x0_sb = xres.tile([128, KD, T], BF16, tag="x0")
            x1_sb = xres.tile([128, KD, T], BF16, tag="x1")

            # ---- router (fp32) ----
            wr_sb = small.tile([128, KD, 2], F32, tag="wr")
            nc.sync.dma_start(wr_sb[:], wr)
            wdiff = small.tile([128, KD, 1], F32, tag="wdiff")
            nc.vector.tensor_sub(wdiff[:], wr_sb[:, :, 0:1], wr_sb[:, :, 1:2])
            rb_sb = small.tile([1, 2], F32, tag="rb")
            nc.sync.dma_start(rb_sb[:], rb)
            bdiff = small.tile([1, 1], F32, tag="bdiff")
            nc.vector.tensor_sub(bdiff[:], rb_sb[:, 0:1], rb_sb[:, 1:2])
            ones_sb = small.tile([1, 128], BF16, tag="ones")
            nc.vector.memset(ones_sb[:], 1.0)
            mask_row = small.tile([1, T], BF16, tag="mrow")
            mask_bc = small.tile([128, T], BF16, tag="mbc")

            prs = [
                psg.tile([1, 512], F32, tag="g", name=f"pr{th}") for th in range(TH)
            ]
            for ko in range(KD):
                xf_t = xf.tile([128, T], F32, tag="xf")
                nc.sync.dma_start(xf_t[:], xt32[ko * 128:(ko + 1) * 128, :])
                for th in range(TH):
                    nc.tensor.matmul(
                        prs[th][:], wdiff[:, ko, :], xf_t[:, ts(th, 512)],
                        start=(ko == 0), stop=(ko == KD - 1),
                    )
            # mask0 = ((l0-l1) + (b0-b1)) >= 0, as 1.0/0.0
            for th in range(TH):
                nc.vector.tensor_scalar(
                    mask_row[:, ts(th, 512)], prs[th][:], bdiff[:], 0.0,
                    mybir.AluOpType.add, mybir.AluOpType.is_ge,
                )
            # broadcast mask row across 128 partitions via K=1 matmul with ones
            for th in range(TH):
                pm = psu.tile([128, 512], F32, tag="u")
                nc.tensor.matmul(
                    pm[:], ones_sb[:], mask_row[:, ts(th, 512)], start=True, stop=True
                )
                nc.vector.tensor_copy(mask_bc[:, ts(th, 512)], pm[:])
            for ko in range(KD):
                nc.vector.tensor_mul(x0_sb[:, ko, :], xtb_sb[:, ko, :], mask_bc[:])
                nc.vector.tensor_sub(x1_sb[:, ko, :], xtb_sb[:, ko, :], x0_sb[:, ko, :])

            # ---- 3 SwiGLU paths ----
            yt_r = yt.rearrange("(md p) t -> p md t", p=128)
            yaccs = [
                yac.tile([128, T], F32, tag=f"yacc{md}", name=f"yacc{md}")
                for md in range(MD)
            ]
            xs_by_path = [xtb_sb, x0_sb, x1_sb]
            for p in range(3):
                xp = xs_by_path[p]
                hch = []
                for mf in range(MF):
                    wg_t = wgp.tile([128, KD, 128], BF16, tag="wg")
                    nc.sync.dma_start(wg_t[:], wgl[p * MF + mf])
                    wu_t = wup.tile([128, KD, 128], BF16, tag="wu")
                    nc.sync.dma_start(wu_t[:], wul[p * MF + mf])
                    h_t = hb.tile([128, T], BF16, tag="h")
                    pgs = [
                        psg.tile([128, 512], F32, tag="g", name=f"pg{th}")
                        for th in range(TH)
                    ]
                    pus = [
                        psu.tile([128, 512], F32, tag="u", name=f"pu{th}")
                        for th in range(TH)
                    ]
                    for th in range(TH):
                        for ko in range(KD):
                            nc.tensor.matmul(
                                pgs[th][:], wg_t[:, ko, :], xp[:, ko, ts(th, 512)],
                                start=(ko == 0), stop=(ko == KD - 1),
                            )
                        for ko in range(KD):
                            nc.tensor.matmul(
                                pus[th][:], wu_t[:, ko, :], xp[:, ko, ts(th, 512)],
                                start=(ko == 0), stop=(ko == KD - 1),
                            )
                    for th in range(TH):
                        pg, pu = pgs[th], pus[th]
                        g_s = gsp.tile([128, 512], BF16, tag="gs")
                        if use_silu:
                            nc.scalar.activation(
                                g_s[:], pg[:], mybir.ActivationFunctionType.Silu
                            )
                        else:
                            # CoreSim lacks Silu; g*sigmoid(g) is identical math
                            nc.scalar.activation(
                                g_s[:], pg[:], mybir.ActivationFunctionType.Sigmoid
                            )
                            nc.vector.tensor_mul(g_s[:], g_s[:], pg[:])
                        nc.vector.tensor_mul(h_t[:, ts(th, 512)], g_s[:], pu[:])
                    hch.append(h_t)
                for md in range(MD):
                    wd_t = wdp.tile([128, KF, 128], BF16, tag="wd")
                    nc.sync.dma_start(wd_t[:], wdl[p * MD + md])
                    pys = [
                        psy.tile([128, 512], F32, tag="y", name=f"py{th}")
                        for th in range(TH)
                    ]
                    for kf in range(KF):
                        for th in range(TH):
                            nc.tensor.matmul(
                                pys[th][:], wd_t[:, kf, :], hch[kf][:, ts(th, 512)],
                                start=(kf == 0), stop=(kf == KF - 1),
                            )
                    for th in range(TH):
                        if p == 0:
                            nc.vector.tensor_copy(
                                yaccs[md][:, ts(th, 512)], pys[th][:]
                            )
                        else:
                            nc.vector.tensor_add(
                                yaccs[md][:, ts(th, 512)],
                                yaccs[md][:, ts(th, 512)],
                                pys[th][:],
                            )
                    if p == 2:
                        # final path: this md slice is complete, ship it out
                        nc.sync.dma_start(yt_r[:, md, :], yaccs[md][:])
    return nc


def build_v3(nc, T=T, D=D, F=F, W=128, use_silu=True,
             psg_b=3, psu_b=2, psy_b=3, w_b=4, hb_extra=0, xf_b=3):
    """v2 + concurrency: the shared expert runs on UNSORTED x (no sort
    dependency) and fills the PE while the sort metadata chain runs; its
    output goes to a separate ysh tensor that the host adds to the
    unpermuted routed output. dest columns come from a DRAM-roundtrip
    broadcast DMA instead of PE transposes.
    """
    KD = D // 128   # k-tiles over D
    MF = F // 128
    MD = D // 128
    KF = F // 128
    TH = T // 512   # 512-token blocks over all tokens (shared path)
    TT = T // 128   # 128-token tiles (for P build / gather)
    half = T // 2
    HB = max(1, half // 512)
    HF = half // HB          # free dim of expert block matmuls (<=512)
    w0 = half - W // 2

    xt32 = nc.dram_tensor("xt32", [D, T], F32, kind="ExternalInput").ap()
    xtok = nc.dram_tensor("xtok", [T, D], BF16, kind="ExternalInput").ap()
    xtb = nc.dram_tensor("xtb", [D, T], BF16, kind="ExternalInput").ap()
    iota = nc.dram_tensor("iota", [1, T], F32, kind="ExternalInput").ap()
    wr = nc.dram_tensor("wr", [128, KD, 2], F32, kind="ExternalInput").ap()
    rb = nc.dram_tensor("rb", [1, 2], F32, kind="ExternalInput").ap()
    wgl = nc.dram_tensor("wgl", [3 * MF, 128, KD, 128], BF16, kind="ExternalInput").ap()
    wul = nc.dram_tensor("wul", [3 * MF, 128, KD, 128], BF16, kind="ExternalInput").ap()
    wdl = nc.dram_tensor("wdl", [3 * MD, 128, KF, 128], BF16, kind="ExternalInput").ap()
    yt = nc.dram_tensor("yt", [D, T], F32, kind="ExternalOutput").ap()
    ysh = nc.dram_tensor("ysh", [D, T], F32, kind="ExternalOutput").ap()
    dscr = nc.dram_tensor("dscr", [1, T], F32).ap()
    dst = nc.dram_tensor("dst", [1, T], F32, kind="ExternalOutput").ap()
    c0o = nc.dram_tensor("c0o", [1, 1], F32, kind="ExternalOutput").ap()

    AF = mybir.ActivationFunctionType

    with tile.TileContext(nc) as tc:
        with (
            tc.tile_pool(name="xres", bufs=1) as xres,
            tc.tile_pool(name="sigp", bufs=1) as sigp,
            tc.tile_pool(name="wg", bufs=w_b) as wgp,
            tc.tile_pool(name="wu", bufs=w_b) as wup,
            tc.tile_pool(name="wd", bufs=w_b) as wdp,
            tc.tile_pool(name="gs", bufs=3) as gsp,
            tc.tile_pool(name="psg", bufs=psg_b, space="PSUM") as psg,
            tc.tile_pool(name="psu", bufs=psu_b, space="PSUM") as psu,
            tc.tile_pool(name="psy", bufs=psy_b, space="PSUM") as psy,
        ):
          AFt = AF

          def silu_into(*a):
              psrc, wdt = a[-2], a[-1]
              g_s = gsp.tile([128, wdt], BF16, tag="gs", name="g_s")
              if use_silu:
                  nc.scalar.activation(g_s[:], psrc[:], AFt.Silu)
              else:
                  nc.scalar.activation(g_s[:], psrc[:], AFt.Sigmoid)
                  nc.vector.tensor_mul(g_s[:], g_s[:], psrc[:])
              return g_s

          with (
            tc.tile_pool(name="xtokp", bufs=1) as xtokp,
            tc.tile_pool(name="xf", bufs=xf_b) as xf,
            tc.tile_pool(name="small", bufs=1) as small,
            tc.tile_pool(name="scr", bufs=3) as scr,
            tc.tile_pool(name="pp", bufs=TT) as pp,
            tc.tile_pool(name="hb", bufs=KF + hb_extra) as hb,
            tc.tile_pool(name="ysp", bufs=1) as ysp,
          ):
            xtok_sb = xtokp.tile([128, TT, D], BF16, tag="xtok")
            xs_sb = xres.tile([128, KD, T], BF16, tag="xs")  # sorted x^T
            # mf=0 weight tiles (0.5MB) lead the queue so the first
            # Ldweights isn't stuck behind the 2MB xtb transfer
            hch = []
            wg_t0 = wgp.tile([128, KD, 128], BF16, tag="wg", name="wg_t0")
            nc.sync.dma_start(wg_t0[:], wgl[0])
            wu_t0 = wup.tile([128, KD, 128], BF16, tag="wu", name="wu_t0")
            nc.sync.dma_start(wu_t0[:], wul[0])
            xtb_sb = xres.tile([128, KD, T], BF16, tag="xtb")
            xtb_r = xtb.rearrange("(ko p) t -> p ko t", p=128)
            for ko in range(KD):
                nc.sync.dma_start(xtb_sb[:, ko, :], xtb_r[:, ko, :])
            h_t0 = hb.tile([128, T], BF16, tag="h", name="h_t0")
            for th in range(TH):
                pg = psg.tile([128, 512], F32, tag="g", name=f"pg0_{th}")
                pu = psu.tile([128, 512], F32, tag="u", name=f"pu0_{th}")
                for ko in range(KD):
                    nc.tensor.matmul(
                        pg[:], wg_t0[:, ko, :], xtb_sb[:, ko, ts(th, 512)],
                        start=(ko == 0), stop=(ko == KD - 1),
                    )
                for ko in range(KD):
                    nc.tensor.matmul(
                        pu[:], wu_t0[:, ko, :], xtb_sb[:, ko, ts(th, 512)],
                        start=(ko == 0), stop=(ko == KD - 1),
                    )
                g_s = silu_into(pg, 512)
                nc.vector.tensor_mul(h_t0[:, ts(th, 512)], g_s[:], pu[:])
            hch.append(h_t0)

            # ---- router (fp32), identical to v1 ----
            wr_sb = small.tile([128, KD, 2], F32, tag="wr")
            nc.sync.dma_start(wr_sb[:], wr)
            wdiff = small.tile([128, KD, 1], F32, tag="wdiff")
            nc.vector.tensor_sub(wdiff[:], wr_sb[:, :, 0:1], wr_sb[:, :, 1:2])
            rb_sb = small.tile([1, 2], F32, tag="rb")
            nc.sync.dma_start(rb_sb[:], rb)
            bdiff = small.tile([1, 1], F32, tag="bdiff")
            nc.vector.tensor_sub(bdiff[:], rb_sb[:, 0:1], rb_sb[:, 1:2])
            mask_row = small.tile([1, T], BF16, tag="mrow")

            prs = [
                psg.tile([1, 512], F32, tag="g", name=f"pr{th}") for th in range(TH)
            ]
            for ko in range(KD):
                xf_t = xf.tile([128, T], F32, tag="xf")
                nc.sync.dma_start(xf_t[:], xt32[ko * 128:(ko + 1) * 128, :])
                for th in range(TH):
                    nc.tensor.matmul(
                        prs[th][:], wdiff[:, ko, :], xf_t[:, ts(th, 512)],
                        start=(ko == 0), stop=(ko == KD - 1),
                    )
            for th in range(TH):
                nc.vector.tensor_scalar(
                    mask_row[:, ts(th, 512)], prs[th][:], bdiff[:], 0.0,
                    mybir.AluOpType.add, mybir.AluOpType.is_ge,
                )

            # ---- sort metadata: dest position per token ----
            iota_sb = small.tile([1, T], F32, tag="iota")
            nc.sync.dma_start(iota_sb[:], iota)
            c0t = small.tile([1, 1], F32, tag="c0t")
            nc.vector.tensor_reduce(
                c0t[:], mask_row[:], mybir.AxisListType.X, mybir.AluOpType.add
            )
            nc.sync.dma_start(c0o, c0t[:])
            zrow = scr.tile([1, T], F32, tag="sc", name="zrow")
            nc.vector.memset(zrow[:], 0.0)
            srow = small.tile([1, T], F32, tag="srow")
            nc.vector.tensor_tensor_scan(
                srow[:], mask_row[:], zrow[:], 0.0,
                mybir.AluOpType.add, mybir.AluOpType.add,
            )
            # dest = m*(s-1) + (1-m)*(c0 + t - s) = B + m*(A - B)
            t1 = scr.tile([1, T], F32, tag="sc", name="t1")
            nc.vector.scalar_tensor_tensor(          # B = (t + c0) - s
                t1[:], iota_sb[:], c0t[:], srow[:],
                mybir.AluOpType.add, mybir.AluOpType.subtract,
            )
            t2 = scr.tile([1, T], F32, tag="sc", name="t2")
            nc.vector.scalar_tensor_tensor(          # A - B = (s - 1) - B
                t2[:], srow[:], -1.0, t1[:],
                mybir.AluOpType.add, mybir.AluOpType.subtract,
            )
            nc.vector.tensor_mul(t2[:], t2[:], mask_row[:])          # m*(A-B)
            dstrow = small.tile([1, T], F32, tag="dstrow")
            nc.vector.tensor_add(dstrow[:], t1[:], t2[:])            # dest row
            nc.sync.dma_start(dst, dstrow[:])
            nc.sync.dma_start(dscr, dstrow[:])
            # dest row -> per-partition columns via DRAM-roundtrip DMA
            dcol = small.tile([128, TT], F32, tag="dcol")
            nc.sync.dma_start(
                dcol[:], dscr.rearrange("o (tt p) -> (o p) tt", p=128)
            )
            ones_f = small.tile([1, 128], F32, tag="onesf")
            nc.vector.memset(ones_f[:], 1.0)
            iota128 = small.tile([128, T], F32, tag="iota128")

            # ---- correction-window signed masks ----
            siga = scr.tile([1, W], F32, tag="sw", name="siga")
            nc.vector.tensor_scalar(
                siga[:], iota_sb[:, w0:w0 + W], c0t[:], None, mybir.AluOpType.is_ge
            )
            sigb = scr.tile([1, W], F32, tag="sw", name="sigb")
            nc.vector.tensor_scalar(
                sigb[:], iota_sb[:, w0:w0 + W], float(half), None,
                mybir.AluOpType.is_ge,
            )
            sigr = small.tile([1, W], F32, tag="sigr")
            nc.vector.tensor_sub(sigr[:], sigb[:], siga[:])   # +/-1/0 for E0 part
            sig_bc = sigp.tile([128, W], BF16, tag="sigbc")
            sgn_bc = sigp.tile([128, W], BF16, tag="sgnbc")

            # shared path over UNSORTED tokens, interleaved with the sort
            # machinery so the PE never waits on the DVE sort chain
            ysh_r = ysh.rearrange("(md p) t -> p md t", p=128)
            ptiles = []
            for mf in range(1, MF):
                wg_t = wgp.tile([128, KD, 128], BF16, tag="wg")
                nc.sync.dma_start(wg_t[:], wgl[mf])
                wu_t = wup.tile([128, KD, 128], BF16, tag="wu")
                nc.sync.dma_start(wu_t[:], wul[mf])
                h_t = hb.tile([128, T], BF16, tag="h")
                for th in range(TH):
                    pg = psg.tile([128, 512], F32, tag="g")
                    pu = psu.tile([128, 512], F32, tag="u")
                    for ko in range(KD):
                        nc.tensor.matmul(
                            pg[:], wg_t[:, ko, :], xtb_sb[:, ko, ts(th, 512)],
                            start=(ko == 0), stop=(ko == KD - 1),
                        )
                    for ko in range(KD):
                        nc.tensor.matmul(
                            pu[:], wu_t[:, ko, :], xtb_sb[:, ko, ts(th, 512)],
                            start=(ko == 0), stop=(ko == KD - 1),
                        )
                    g_s = silu_into(pg, 512)
                    nc.vector.tensor_mul(h_t[:, ts(th, 512)], g_s[:], pu[:])
                hch.append(h_t)
                # interleaved sort machinery (DVE slack during shared phase)
                if mf == 1:
                    for th in range(TH):
                        pm = psy.tile([128, 512], F32, tag="y", name=f"pio{th}")
                        nc.tensor.matmul(
                            pm[:], ones_f[:], iota_sb[:, ts(th, 512)],
                            start=True, stop=True,
                        )
                        nc.vector.tensor_copy(iota128[:, ts(th, 512)], pm[:])
                if 2 <= mf <= TT // 2 + 1:
                    for tt in (2 * (mf - 2), 2 * (mf - 2) + 1):
                        p_t = pp.tile([128, T], BF16, tag="p", name=f"P{tt}")
                        nc.vector.tensor_scalar(
                            p_t[:], iota128[:], dcol[:, tt:tt + 1], None,
                            mybir.AluOpType.is_equal,
                        )
                        ptiles.append(p_t)
                if mf == min(TT // 2 + 2, MF - 1):
                    psig = psu.tile([128, W], F32, tag="u", name="psig")
                    nc.tensor.matmul(
                        psig[:], ones_f[:], sigr[:], start=True, stop=True
                    )
                    nc.vector.tensor_copy(sig_bc[:], psig[:])
                    nc.vector.tensor_scalar_mul(sgn_bc[:], sig_bc[:], -1.0)
            for md in range(MD):
                wd_t = wdp.tile([128, KF, 128], BF16, tag="wd")
                nc.sync.dma_start(wd_t[:], wdl[md])
                ys_t = ysp.tile([128, T], F32, tag=f"ysh{md}", name=f"ysh{md}")
                for th in range(TH):
                    py = psy.tile([128, 512], F32, tag="y")
                    for kf in range(KF):
                        nc.tensor.matmul(
                            py[:], wd_t[:, kf, :], hch[kf][:, ts(th, 512)],
                            start=(kf == 0), stop=(kf == KF - 1),
                        )
                    nc.vector.tensor_copy(ys_t[:, ts(th, 512)], py[:])
                nc.sync.dma_start(ysh_r[:, md, :], ys_t[:])

            # ---- gather matmuls: xs = x_tok^T @ P ----
            # token-major strips deferred: only needed here (~170us in),
            # keeps the kernel-start DMA queue short (trace: 16us first gap)
            for tt in range(TT):
                nc.sync.dma_start(
                    xtok_sb[:, tt, :], xtok[tt * 128:(tt + 1) * 128, :]
                )
            for dt in range(KD):
                for th in range(TH):
                    px = psg.tile([128, 512], F32, tag="g", name=f"px{dt}_{th}")
                    for tt in range(TT):
                        nc.tensor.matmul(
                            px[:], xtok_sb[:, tt, ts(dt, 128)],
                            ptiles[tt][:, ts(th, 512)],
                            start=(tt == 0), stop=(tt == TT - 1),
                        )
                    nc.vector.tensor_copy(xs_sb[:, dt, ts(th, 512)], px[:])

          # ---- routed expert phase (shared/sort pools closed) ----
          with (
            tc.tile_pool(name="hh", bufs=KF + 1) as hhp,
            tc.tile_pool(name="hw", bufs=KF + 1) as hwp,
            tc.tile_pool(name="yac", bufs=1) as yac,
          ):
            yt_r = yt.rearrange("(md p) t -> p md t", p=128)
            yaccs = [
                yac.tile([128, T], F32, tag=f"yacc{md}", name=f"yacc{md}")
                for md in range(MD)
            ]
            for md in range(MD):
                nc.gpsimd.memset(yaccs[md][:], 0.0)

            # expert blocks + correction window
            for e in (1, 2):
                off = 0 if e == 1 else half
                wmask = sig_bc if e == 1 else sgn_bc
                hA = []
                hW = []
                for mf in range(MF):
                    wg_t = wgp.tile([128, KD, 128], BF16, tag="wg")
                    nc.sync.dma_start(wg_t[:], wgl[e * MF + mf])
                    wu_t = wup.tile([128, KD, 128], BF16, tag="wu")
                    nc.sync.dma_start(wu_t[:], wul[e * MF + mf])
                    hA_t = hhp.tile([128, half], BF16, tag="hh")
                    for hbk in range(HB):
                        o2 = off + hbk * HF
                        pg = psg.tile([128, HF], F32, tag="g")
                        pu = psu.tile([128, HF], F32, tag="u")
                        for ko in range(KD):
                            nc.tensor.matmul(
                                pg[:], wg_t[:, ko, :], xs_sb[:, ko, o2:o2 + HF],
                                start=(ko == 0), stop=(ko == KD - 1),
                            )
                        for ko in range(KD):
                            nc.tensor.matmul(
                                pu[:], wu_t[:, ko, :], xs_sb[:, ko, o2:o2 + HF],
                                start=(ko == 0), stop=(ko == KD - 1),
                            )
                        g_s = silu_into(hA_t, pg, HF)
                        nc.vector.tensor_mul(
                            hA_t[:, hbk * HF:(hbk + 1) * HF], g_s[:], pu[:]
                        )
                    # correction window: the in-block half of this expert's
                    # window H is already in hA_t; only compute the
                    # out-of-block half (Wh columns) with fresh matmuls.
                    Wh = W // 2
                    oo = half if e == 1 else w0      # out-of-block global cols
                    ob = (Wh, W) if e == 1 else (0, Wh)   # pos within window
                    ib = (0, Wh) if e == 1 else (Wh, W)
                    ib_lo = w0 if e == 1 else 0           # block-local offset
                    hW_t = hwp.tile([128, W], BF16, tag="hw")
                    pgw = psg.tile([128, Wh], F32, tag="g", name="pgw")
                    puw = psu.tile([128, Wh], F32, tag="u", name="puw")
                    for ko in range(KD):
                        nc.tensor.matmul(
                            pgw[:], wg_t[:, ko, :], xs_sb[:, ko, oo:oo + Wh],
                            start=(ko == 0), stop=(ko == KD - 1),
                        )
                    for ko in range(KD):
                        nc.tensor.matmul(
                            puw[:], wu_t[:, ko, :], xs_sb[:, ko, oo:oo + Wh],
                            start=(ko == 0), stop=(ko == KD - 1),
                        )
                    g_s = silu_into(hW_t, pgw, Wh)
                    nc.vector.tensor_mul(hW_t[:, ob[0]:ob[1]], g_s[:], puw[:])
                    nc.vector.tensor_mul(
                        hW_t[:, ob[0]:ob[1]], hW_t[:, ob[0]:ob[1]],
                        wmask[:, ob[0]:ob[1]],
                    )
                    nc.vector.tensor_mul(
                        hW_t[:, ib[0]:ib[1]], hA_t[:, ib_lo:ib_lo + Wh],
                        wmask[:, ib[0]:ib[1]],
                    )
                    hA.append(hA_t)
                    hW.append(hW_t)
                for md in range(MD):
                    wd_t = wdp.tile([128, KF, 128], BF16, tag="wd")
                    nc.sync.dma_start(wd_t[:], wdl[e * MD + md])
                    for hbk in range(HB):
                        o2 = off + hbk * HF
                        py = psy.tile([128, HF], F32, tag="y")
                        for kf in range(KF):
                            nc.tensor.matmul(
                                py[:], wd_t[:, kf, :],
                                hA[kf][:, hbk * HF:(hbk + 1) * HF],
                                start=(kf == 0), stop=(kf == KF - 1),
                            )
                        nc.vector.tensor_add(
                            yaccs[md][:, o2:o2 + HF],
                            yaccs[md][:, o2:o2 + HF], py[:],
                        )
                    pyw = psy.tile([128, W], F32, tag="y", name="pyw")
                    for kf in range(KF):
                        nc.tensor.matmul(
                            pyw[:], wd_t[:, kf, :], hW[kf][:],
                            start=(kf == 0), stop=(kf == KF - 1),
                        )
                    nc.vector.tensor_add(
                        yaccs[md][:, w0:w0 + W],
                        yaccs[md][:, w0:w0 + W], pyw[:],
                    )
                    if e == 2:
                        nc.sync.dma_start(yt_r[:, md, :], yaccs[md][:])
    return nc


def build_v2(nc, T=T, D=D, F=F, W=128, use_silu=True,
             psg_b=3, psu_b=2, psy_b=3, w_b=4, hb_extra=2, xf_b=3):
    """Token-sorted variant: sort tokens by routed expert (permutation-matrix
    matmul), run expert0 on sorted block [0, T/2) and expert1 on [T/2, T)
    unmasked, and fix the misassigned span around T/2 with a signed-mask
    correction window of W tokens. Exports dst (sort positions) and c0
    (expert-0 count) so the host can unpermute / verify window coverage.
    """
    KD = D // 128   # k-tiles over D
    MF = F // 128
    MD = D // 128
    KF = F // 128
    TH = T // 512   # 512-token blocks over all tokens (shared path)
    TT = T // 128   # 128-token tiles (for P build / gather)
    half = T // 2
    HB = max(1, half // 512)
    HF = half // HB          # free dim of expert block matmuls (<=512)
    w0 = half - W // 2

    xt32 = nc.dram_tensor("xt32", [D, T], F32, kind="ExternalInput").ap()
    xtok = nc.dram_tensor("xtok", [T, D], BF16, kind="ExternalInput").ap()
    iota = nc.dram_tensor("iota", [1, T], F32, kind="ExternalInput").ap()
    wr = nc.dram_tensor("wr", [128, KD, 2], F32, kind="ExternalInput").ap()
    rb = nc.dram_tensor("rb", [1, 2], F32, kind="ExternalInput").ap()
    wgl = nc.dram_tensor("wgl", [3 * MF, 128, KD, 128], BF16, kind="ExternalInput").ap()
    wul = nc.dram_tensor("wul", [3 * MF, 128, KD, 128], BF16, kind="ExternalInput").ap()
    wdl = nc.dram_tensor("wdl", [3 * MD, 128, KF, 128], BF16, kind="ExternalInput").ap()
    yt = nc.dram_tensor("yt", [D, T], F32, kind="ExternalOutput").ap()
    dst = nc.dram_tensor("dst", [1, T], F32, kind="ExternalOutput").ap()
    c0o = nc.dram_tensor("c0o", [1, 1], F32, kind="ExternalOutput").ap()

    AF = mybir.ActivationFunctionType

    with tile.TileContext(nc) as tc:
        with (
            tc.tile_pool(name="xres", bufs=1) as xres,
            tc.tile_pool(name="sigp", bufs=1) as sigp,
            tc.tile_pool(name="psg", bufs=psg_b, space="PSUM") as psg,
            tc.tile_pool(name="psu", bufs=psu_b, space="PSUM") as psu,
            tc.tile_pool(name="psy", bufs=psy_b, space="PSUM") as psy,
        ):
          with (
            tc.tile_pool(name="xtokp", bufs=1) as xtokp,
            tc.tile_pool(name="xf", bufs=xf_b) as xf,
            tc.tile_pool(name="small", bufs=1) as small,
            tc.tile_pool(name="scr", bufs=3) as scr,
            tc.tile_pool(name="pp", bufs=TT) as pp,
          ):
            # token-major x strips (gather lhsT)
            xtok_sb = xtokp.tile([128, TT, D], BF16, tag="xtok")
            for tt in range(TT):
                nc.sync.dma_start(
                    xtok_sb[:, tt, :], xtok[tt * 128:(tt + 1) * 128, :]
                )
            xs_sb = xres.tile([128, KD, T], BF16, tag="xs")  # sorted x^T

            # shared-path mf=0 first: its inputs (xtb, wg0, wu0) lead the
            # DMA queue, so the PE starts ~3us earlier than router-first
            hch = []
            wg_t0 = wgp.tile([128, KD, 128], BF16, tag="wg", name="wg_t0")
            nc.sync.dma_start(wg_t0[:], wgl[0])
            wu_t0 = wup.tile([128, KD, 128], BF16, tag="wu", name="wu_t0")
            nc.sync.dma_start(wu_t0[:], wul[0])
            h_t0 = hb.tile([128, T], BF16, tag="h", name="h_t0")
            for th in range(TH):
                pg = psg.tile([128, 512], F32, tag="g", name=f"pg0_{th}")
                pu = psu.tile([128, 512], F32, tag="u", name=f"pu0_{th}")
                for ko in range(KD):
                    nc.tensor.matmul(
                        pg[:], wg_t0[:, ko, :], xtb_sb[:, ko, ts(th, 512)],
                        start=(ko == 0), stop=(ko == KD - 1),
                    )
                for ko in range(KD):
                    nc.tensor.matmul(
                        pu[:], wu_t0[:, ko, :], xtb_sb[:, ko, ts(th, 512)],
                        start=(ko == 0), stop=(ko == KD - 1),
                    )
                g_s = silu_into(pg, 512)
                nc.vector.tensor_mul(h_t0[:, ts(th, 512)], g_s[:], pu[:])
            hch.append(h_t0)

            # ---- router (fp32), identical to v1 ----
            wr_sb = small.tile([128, KD, 2], F32, tag="wr")
            nc.sync.dma_start(wr_sb[:], wr)
            wdiff = small.tile([128, KD, 1], F32, tag="wdiff")
            nc.vector.tensor_sub(wdiff[:], wr_sb[:, :, 0:1], wr_sb[:, :, 1:2])
            rb_sb = small.tile([1, 2], F32, tag="rb")
            nc.sync.dma_start(rb_sb[:], rb)
            bdiff = small.tile([1, 1], F32, tag="bdiff")
            nc.vector.tensor_sub(bdiff[:], rb_sb[:, 0:1], rb_sb[:, 1:2])
            mask_row = small.tile([1, T], BF16, tag="mrow")

            prs = [
                psg.tile([1, 512], F32, tag="g", name=f"pr{th}") for th in range(TH)
            ]
            for ko in range(KD):
                xf_t = xf.tile([128, T], F32, tag="xf")
                nc.sync.dma_start(xf_t[:], xt32[ko * 128:(ko + 1) * 128, :])
                for th in range(TH):
                    nc.tensor.matmul(
                        prs[th][:], wdiff[:, ko, :], xf_t[:, ts(th, 512)],
                        start=(ko == 0), stop=(ko == KD - 1),
                    )
            for th in range(TH):
                nc.vector.tensor_scalar(
                    mask_row[:, ts(th, 512)], prs[th][:], bdiff[:], 0.0,
                    mybir.AluOpType.add, mybir.AluOpType.is_ge,
                )

            # ---- sort metadata: dest position per token ----
            iota_sb = small.tile([1, T], F32, tag="iota")
            nc.sync.dma_start(iota_sb[:], iota)
            c0t = small.tile([1, 1], F32, tag="c0t")
            nc.vector.tensor_reduce(
                c0t[:], mask_row[:], mybir.AxisListType.X, mybir.AluOpType.add
            )
            nc.sync.dma_start(c0o, c0t[:])
            zrow = scr.tile([1, T], F32, tag="sc", name="zrow")
            nc.vector.memset(zrow[:], 0.0)
            srow = small.tile([1, T], F32, tag="srow")
            nc.vector.tensor_tensor_scan(
                srow[:], mask_row[:], zrow[:], 0.0,
                mybir.AluOpType.add, mybir.AluOpType.add,
            )
            # dest = m*(s-1) + (1-m)*(c0 + t - s) = B + m*(A - B)
            t1 = scr.tile([1, T], F32, tag="sc", name="t1")
            nc.vector.scalar_tensor_tensor(          # B = (t + c0) - s
                t1[:], iota_sb[:], c0t[:], srow[:],
                mybir.AluOpType.add, mybir.AluOpType.subtract,
            )
            t2 = scr.tile([1, T], F32, tag="sc", name="t2")
            nc.vector.scalar_tensor_tensor(          # A - B = (s - 1) - B
                t2[:], srow[:], -1.0, t1[:],
                mybir.AluOpType.add, mybir.AluOpType.subtract,
            )
            nc.vector.tensor_mul(t2[:], t2[:], mask_row[:])          # m*(A-B)
            dtile = small.tile([128, T], F32, tag="dtile")
            nc.vector.memset(dtile[:], 0.0)
            nc.vector.tensor_add(dtile[0:1, :], t1[:], t2[:])        # dest row
            nc.sync.dma_start(dst, dtile[0:1, :])

            # ---- dest row -> per-partition columns (PE transpose) ----
            ident = small.tile([128, 128], F32, tag="ident")
            make_identity(nc, ident[:])
            dcol = small.tile([128, TT], F32, tag="dcol")
            for tt in range(TT):
                ptp = psy.tile([128, 128], F32, tag="y", name=f"ptp{tt}")
                nc.tensor.transpose(ptp[:], dtile[:, ts(tt, 128)], ident[:])
                nc.vector.tensor_copy(dcol[:, tt:tt + 1], ptp[:, 0:1])

            # ---- iota broadcast across partitions ----
            ones_f = small.tile([1, 128], F32, tag="onesf")
            nc.vector.memset(ones_f[:], 1.0)
            iota128 = small.tile([128, T], F32, tag="iota128")
            for th in range(TH):
                pm = psy.tile([128, 512], F32, tag="y", name=f"pio{th}")
                nc.tensor.matmul(
                    pm[:], ones_f[:], iota_sb[:, ts(th, 512)], start=True, stop=True
                )
                nc.vector.tensor_copy(iota128[:, ts(th, 512)], pm[:])

            # ---- permutation tiles + gather matmuls: xs = x_tok^T @ P ----
            ptiles = []
            for tt in range(TT):
                p_t = pp.tile([128, T], BF16, tag="p", name=f"P{tt}")
                nc.vector.tensor_scalar(
                    p_t[:], iota128[:], dcol[:, tt:tt + 1], None,
                    mybir.AluOpType.is_equal,
                )
                ptiles.append(p_t)
            for dt in range(KD):
                for th in range(TH):
                    px = psg.tile([128, 512], F32, tag="g", name=f"px{dt}_{th}")
                    for tt in range(TT):
                        nc.tensor.matmul(
                            px[:], xtok_sb[:, tt, ts(dt, 128)],
                            ptiles[tt][:, ts(th, 512)],
                            start=(tt == 0), stop=(tt == TT - 1),
                        )
                    nc.vector.tensor_copy(xs_sb[:, dt, ts(th, 512)], px[:])

            # ---- correction-window signed masks ----
            siga = scr.tile([1, W], F32, tag="sw", name="siga")
            nc.vector.tensor_scalar(
                siga[:], iota_sb[:, w0:w0 + W], c0t[:], None, mybir.AluOpType.is_ge
            )
            sigb = scr.tile([1, W], F32, tag="sw", name="sigb")
            nc.vector.tensor_scalar(
                sigb[:], iota_sb[:, w0:w0 + W], float(half), None,
                mybir.AluOpType.is_ge,
            )
            sigr = scr.tile([1, W], F32, tag="sw", name="sigr")
            nc.vector.tensor_sub(sigr[:], sigb[:], siga[:])   # +/-1/0 for E0 part
            sig_bc = sigp.tile([128, W], BF16, tag="sigbc")
            sgn_bc = sigp.tile([128, W], BF16, tag="sgnbc")
            psig = psu.tile([128, W], F32, tag="u", name="psig")
            nc.tensor.matmul(psig[:], ones_f[:], sigr[:], start=True, stop=True)
            nc.vector.tensor_copy(sig_bc[:], psig[:])
            nc.vector.tensor_scalar_mul(sgn_bc[:], sig_bc[:], -1.0)

          # ---- paths (sort-phase pools closed; open main-phase pools) ----
          with (
            tc.tile_pool(name="wg", bufs=w_b) as wgp,
            tc.tile_pool(name="wu", bufs=w_b) as wup,
            tc.tile_pool(name="wd", bufs=w_b) as wdp,
            tc.tile_pool(name="hb", bufs=KF + hb_extra) as hb,
            tc.tile_pool(name="hh", bufs=KF + 1) as hhp,
            tc.tile_pool(name="hw", bufs=KF + 1) as hwp,
            tc.tile_pool(name="gs", bufs=3) as gsp,
            tc.tile_pool(name="yac", bufs=1) as yac,
          ):
            yt_r = yt.rearrange("(md p) t -> p md t", p=128)
            yaccs = [
                yac.tile([128, T], F32, tag=f"yacc{md}", name=f"yacc{md}")
                for md in range(MD)
            ]

            def silu_into(dstp, psrc, wdt):
                g_s = gsp.tile([128, wdt], BF16, tag="gs", name="g_s")
                if use_silu:
                    nc.scalar.activation(g_s[:], psrc[:], AF.Silu)
                else:
                    nc.scalar.activation(g_s[:], psrc[:], AF.Sigmoid)
                    nc.vector.tensor_mul(g_s[:], g_s[:], psrc[:])
                return g_s

            # shared path over all (sorted) tokens
            hch = []
            for mf in range(MF):
                wg_t = wgp.tile([128, KD, 128], BF16, tag="wg")
                nc.sync.dma_start(wg_t[:], wgl[mf])
                wu_t = wup.tile([128, KD, 128], BF16, tag="wu")
                nc.sync.dma_start(wu_t[:], wul[mf])
                h_t = hb.tile([128, T], BF16, tag="h")
                for th in range(TH):
                    pg = psg.tile([128, 512], F32, tag="g")
                    pu = psu.tile([128, 512], F32, tag="u")
                    for ko in range(KD):
                        nc.tensor.matmul(
                            pg[:], wg_t[:, ko, :], xs_sb[:, ko, ts(th, 512)],
                            start=(ko == 0), stop=(ko == KD - 1),
                        )
                    for ko in range(KD):
                        nc.tensor.matmul(
                            pu[:], wu_t[:, ko, :], xs_sb[:, ko, ts(th, 512)],
                            start=(ko == 0), stop=(ko == KD - 1),
                        )
                    g_s = silu_into(h_t, pg, 512)
                    nc.vector.tensor_mul(h_t[:, ts(th, 512)], g_s[:], pu[:])
                hch.append(h_t)
            for md in range(MD):
                wd_t = wdp.tile([128, KF, 128], BF16, tag="wd")
                nc.sync.dma_start(wd_t[:], wdl[md])
                for th in range(TH):
                    py = psy.tile([128, 512], F32, tag="y")
                    for kf in range(KF):
                        nc.tensor.matmul(
                            py[:], wd_t[:, kf, :], hch[kf][:, ts(th, 512)],
                            start=(kf == 0), stop=(kf == KF - 1),
                        )
                    nc.vector.tensor_copy(yaccs[md][:, ts(th, 512)], py[:])

            # expert blocks + correction window
            for e in (1, 2):
                off = 0 if e == 1 else half
                wmask = sig_bc if e == 1 else sgn_bc
                hA = []
                hW = []
                for mf in range(MF):
                    wg_t = wgp.tile([128, KD, 128], BF16, tag="wg")
                    nc.sync.dma_start(wg_t[:], wgl[e * MF + mf])
                    wu_t = wup.tile([128, KD, 128], BF16, tag="wu")
                    nc.sync.dma_start(wu_t[:], wul[e * MF + mf])
                    hA_t = hhp.tile([128, half], BF16, tag="hh")
                    for hbk in range(HB):
                        o2 = off + hbk * HF
                        pg = psg.tile([128, HF], F32, tag="g")
                        pu = psu.tile([128, HF], F32, tag="u")
                        for ko in range(KD):
                            nc.tensor.matmul(
                                pg[:], wg_t[:, ko, :], xs_sb[:, ko, o2:o2 + HF],
                                start=(ko == 0), stop=(ko == KD - 1),
                            )
                        for ko in range(KD):
                            nc.tensor.matmul(
                                pu[:], wu_t[:, ko, :], xs_sb[:, ko, o2:o2 + HF],
                                start=(ko == 0), stop=(ko == KD - 1),
                            )
                        g_s = silu_into(hA_t, pg, HF)
                        nc.vector.tensor_mul(
                            hA_t[:, hbk * HF:(hbk + 1) * HF], g_s[:], pu[:]
                        )
                    # correction window: the in-block half of this expert's
                    # window H is already in hA_t; only compute the
                    # out-of-block half (Wh columns) with fresh matmuls.
                    Wh = W // 2
                    oo = half if e == 1 else w0      # out-of-block global cols
                    ob = (Wh, W) if e == 1 else (0, Wh)   # pos within window
                    ib = (0, Wh) if e == 1 else (Wh, W)
                    ib_lo = w0 if e == 1 else 0           # block-local offset
                    hW_t = hwp.tile([128, W], BF16, tag="hw")
                    pgw = psg.tile([128, Wh], F32, tag="g", name="pgw")
                    puw = psu.tile([128, Wh], F32, tag="u", name="puw")
                    for ko in range(KD):
                        nc.tensor.matmul(
                            pgw[:], wg_t[:, ko, :], xs_sb[:, ko, oo:oo + Wh],
                            start=(ko == 0), stop=(ko == KD - 1),
                        )
                    for ko in range(KD):
                        nc.tensor.matmul(
                            puw[:], wu_t[:, ko, :], xs_sb[:, ko, oo:oo + Wh],
                            start=(ko == 0), stop=(ko == KD - 1),
                        )
                    g_s = silu_into(hW_t, pgw, Wh)
                    nc.vector.tensor_mul(hW_t[:, ob[0]:ob[1]], g_s[:], puw[:])
                    nc.vector.tensor_mul(
                        hW_t[:, ob[0]:ob[1]], hW_t[:, ob[0]:ob[1]],
                        wmask[:, ob[0]:ob[1]],
                    )
                    nc.vector.tensor_mul(
                        hW_t[:, ib[0]:ib[1]], hA_t[:, ib_lo:ib_lo + Wh],
                        wmask[:, ib[0]:ib[1]],
                    )
                    hA.append(hA_t)
                    hW.append(hW_t)
                for md in range(MD):
                    wd_t = wdp.tile([128, KF, 128], BF16, tag="wd")
                    nc.sync.dma_start(wd_t[:], wdl[e * MD + md])
                    for hbk in range(HB):
                        o2 = off + hbk * HF
                        py = psy.tile([128, HF], F32, tag="y")
                        for kf in range(KF):
                            nc.tensor.matmul(
                                py[:], wd_t[:, kf, :],
                                hA[kf][:, hbk * HF:(hbk + 1) * HF],
                                start=(kf == 0), stop=(kf == KF - 1),
                            )
                        nc.vector.tensor_add(
                            yaccs[md][:, o2:o2 + HF],
                            yaccs[md][:, o2:o2 + HF], py[:],
                        )
                    pyw = psy.tile([128, W], F32, tag="y", name="pyw")
                    for kf in range(KF):
                        nc.tensor.matmul(
                            pyw[:], wd_t[:, kf, :], hW[kf][:],
                            start=(kf == 0), stop=(kf == KF - 1),
                        )
                    nc.vector.tensor_add(
                        yaccs[md][:, w0:w0 + W],
                        yaccs[md][:, w0:w0 + W], pyw[:],
                    )
                    if e == 2:
                        nc.sync.dma_start(yt_r[:, md, :], yaccs[md][:])
    return nc


def pack_inputs(x, W_router, router_bias, Wg, Wu, Wd, Sg, Su, Sd, T=T, D=D, F=F):
    """Host-side sharding + layout prep. Returns per-core in_maps."""
    KD, MF, MD, KF = D // 128, F // 128, D // 128, F // 128
    flat = np.asarray(x, np.float32).reshape(-1, D)
    n_tokens = flat.shape[0]
    assert n_tokens == N_CORES * T
    xt = np.ascontiguousarray(flat.T)  # [D, N]
    xtb_full = xt.astype(ml_dtypes.bfloat16)

    G = np.stack([np.asarray(Sg), np.asarray(Wg)[0], np.asarray(Wg)[1]]).astype(np.float32)
    U = np.stack([np.asarray(Su), np.asarray(Wu)[0], np.asarray(Wu)[1]]).astype(np.float32)
    Dn = np.stack([np.asarray(Sd), np.asarray(Wd)[0], np.asarray(Wd)[1]]).astype(np.float32)
    wgl = np.ascontiguousarray(
        G.reshape(3, KD, 128, MF, 128).transpose(0, 3, 2, 1, 4)
    ).reshape(3 * MF, 128, KD, 128).astype(ml_dtypes.bfloat16)
    wul = np.ascontiguousarray(
        U.reshape(3, KD, 128, MF, 128).transpose(0, 3, 2, 1, 4)
    ).reshape(3 * MF, 128, KD, 128).astype(ml_dtypes.bfloat16)
    wdl = np.ascontiguousarray(
        Dn.reshape(3, KF, 128, MD, 128).transpose(0, 3, 2, 1, 4)
    ).reshape(3 * MD, 128, KF, 128).astype(ml_dtypes.bfloat16)
    wr_h = np.ascontiguousarray(
        np.asarray(W_router, np.float32).reshape(KD, 128, 2).transpose(1, 0, 2)
    )
    rb_h = np.asarray(router_bias, np.float32).reshape(1, 2)

    in_maps = []
    for c in range(N_CORES):
        sl = slice(c * T, (c + 1) * T)
        in_maps.append({
            "xt32": np.ascontiguousarray(xt[:, sl]),
            "xtb": np.ascontiguousarray(xtb_full[:, sl]),
            "wr": wr_h,
            "rb": rb_h,
            "wgl": wgl,
            "wul": wul,
            "wdl": wdl,
        })
    return in_maps


WINDOW = 96


def pack_inputs_v2(x, W_router, router_bias, Wg, Wu, Wd, Sg, Su, Sd, T=T, D=D, F=F):
    base = pack_inputs(x, W_router, router_bias, Wg, Wu, Wd, Sg, Su, Sd, T, D, F)
    flat = np.asarray(x, np.float32).reshape(-1, D)
    flat_b = flat.astype(ml_dtypes.bfloat16)
    iota_row = np.arange(T, dtype=np.float32).reshape(1, T)
    in_maps = []
    for c, m in enumerate(base):
        m = dict(m)
        m["xtok"] = np.ascontiguousarray(flat_b[c * T:(c + 1) * T, :])
        m["iota"] = iota_row
        in_maps.append(m)
    return in_maps


_CACHE = {}


def _get_compiled(ver="v2"):
    key = f"nc_{ver}"
    if key not in _CACHE:
        nc = bacc.Bacc(
            "TRN2",
            target_bir_lowering=False,
            # axon clients cannot host a BassDebugger; native path can
            debug=not axon_active(),
            num_devices=N_CORES,
        )
        if ver == "v3":
            build_v3(nc, W=WINDOW)
        elif ver == "v2":
            build_v2(nc, W=WINDOW)
        else:
            build(nc)
        nc.compile()
        _CACHE[key] = nc
    return _CACHE[key]


def _run_v1(np_args, x_shape, _trace=False):
    nc = _get_compiled("v1")
    in_maps = pack_inputs(*np_args)
    res = run_bass_kernel_spmd(
        nc, in_maps, core_ids=list(range(N_CORES)), trace=_trace
    )
    out_t = np.concatenate(
        [res.results[c]["yt"] for c in range(N_CORES)], axis=1
    )
    if _trace:
        _CACHE["last_result"] = res
    return np.ascontiguousarray(out_t.T).reshape(x_shape).astype(np.float32)


def kernel(x, W_router, router_bias, Wg, Wu, Wd, Sg, Su, Sd, _trace=False, **_kw):
    np_args = (x, W_router, router_bias, Wg, Wu, Wd, Sg, Su, Sd)
    x_shape = np.asarray(x).shape
    nc = _get_compiled("v3")
    in_maps = pack_inputs_v2(*np_args)
    res = run_bass_kernel_spmd(
        nc, in_maps, core_ids=list(range(N_CORES)), trace=_trace
    )
    half, w0 = T // 2, T // 2 - WINDOW // 2
    cols = []
    for c in range(N_CORES):
        c0 = int(round(float(res.results[c]["c0o"][0, 0])))
        if not (w0 <= c0 <= w0 + WINDOW):
            # expert split fell outside the static correction window
            # (~8-sigma event for these inputs): rerun with the dense kernel
            return _run_v1(np_args, x_shape, _trace)
        dest = np.rint(res.results[c]["dst"][0]).astype(np.int64)
        # routed output is in sorted order; shared output is unsorted
        cols.append(res.results[c]["ysh"] + res.results[c]["yt"][:, dest])
    out_t = np.concatenate(cols, axis=1)  # [D, N]
    if _trace:
        _CACHE["last_result"] = res
    return np.ascontiguousarray(out_t.T).reshape(x_shape).astype(np.float32)



# revision 2
# speedup vs baseline: 1.1219x; 1.1219x over previous
import os
import sys

for _p in ("/opt/trn_rl_repo", "/root/.axon_site/_ro/trn_rl_repo"):
    if os.path.isdir(_p) and _p not in sys.path:
        sys.path.insert(0, _p)

import numpy as np
import ml_dtypes

import concourse.bass as bass
import concourse.tile as tile
import concourse.mybir as mybir
from concourse import bacc
from concourse._compat import axon_active
from concourse.bass import ts
from concourse.bass_utils import run_bass_kernel_spmd

N_CORES = 8
D = 1024
F = 2048
T = 1024  # tokens per core (8192 / 8)

BF16 = mybir.dt.bfloat16
F32 = mybir.dt.float32


def build(nc, T=T, D=D, F=F, use_silu=True, psg_b=3, psu_b=3, psy_b=2,
          w_b=4, hb_extra=6, xf_b=3):
    """Dense fallback: per-core MoE FFN with on-device router + masked paths.

    Layout: activations transposed (feature on partitions, tokens on free dim).
    Paths: [shared, expert0, expert1]; expert token masks folded into the
    input (x0 = x*m0, x1 = x - x0) so all three paths sum directly.
    """
    KD = D // 128   # k-tiles over D (gate/up contraction, also out tiles of down)
    MF = F // 128   # m-tiles over F
    MD = D // 128
    KF = F // 128
    TH = T // 512   # 512-token free-dim blocks

    xt32 = nc.dram_tensor("xt32", [D, T], F32, kind="ExternalInput").ap()
    xtb = nc.dram_tensor("xtb", [D, T], BF16, kind="ExternalInput").ap()
    wr = nc.dram_tensor("wr", [128, KD, 2], F32, kind="ExternalInput").ap()
    rb = nc.dram_tensor("rb", [1, 2], F32, kind="ExternalInput").ap()
    wgl = nc.dram_tensor("wgl", [3 * MF, 128, KD, 128], BF16, kind="ExternalInput").ap()
    wul = nc.dram_tensor("wul", [3 * MF, 128, KD, 128], BF16, kind="ExternalInput").ap()
    wdl = nc.dram_tensor("wdl", [3 * MD, 128, KF, 128], BF16, kind="ExternalInput").ap()
    yt = nc.dram_tensor("yt", [D, T], F32, kind="ExternalOutput").ap()

    with tile.TileContext(nc) as tc:
        with (
            tc.tile_pool(name="xres", bufs=1) as xres,
            tc.tile_pool(name="xf", bufs=xf_b) as xf,
            tc.tile_pool(name="small", bufs=1) as small,
            tc.tile_pool(name="wg", bufs=w_b) as wgp,
            tc.tile_pool(name="wu", bufs=w_b) as wup,
            tc.tile_pool(name="wd", bufs=w_b) as wdp,
            tc.tile_pool(name="hb", bufs=KF + hb_extra) as hb,
            tc.tile_pool(name="gs", bufs=3) as gsp,
            tc.tile_pool(name="yac", bufs=1) as yac,
            tc.tile_pool(name="psg", bufs=psg_b, space="PSUM") as psg,
            tc.tile_pool(name="psu", bufs=psu_b, space="PSUM") as psu,
            tc.tile_pool(name="psy", bufs=psy_b, space="PSUM") as psy,
        ):
            # resident transposed input (bf16) + masked variants
            xtb_sb = xres.tile([128, KD, T], BF16, tag="xtb")
            xtb_r = xtb.rearrange("(ko p) t -> p ko t", p=128)
            for ko in range(KD):
                nc.sync.dma_start(xtb_sb[:, ko, :], xtb_r[:, ko, :])
            x0_sb = xres.tile([128, KD, T], BF16, tag="x0")
            x1_sb = xres.tile([128, KD, T], BF16, tag="x1")

            # ---- router (fp32) ----
            wr_sb = small.tile([128, KD, 2], F32, tag="wr")
            nc.sync.dma_start(wr_sb[:], wr)
            wdiff = small.tile([128, KD, 1], F32, tag="wdiff")
            nc.vector.tensor_sub(wdiff[:], wr_sb[:, :, 0:1], wr_sb[:, :, 1:2])
            rb_sb = small.tile([1, 2], F32, tag="rb")
            nc.sync.dma_start(rb_sb[:], rb)
            bdiff = small.tile([1, 1], F32, tag="bdiff")
            nc.vector.tensor_sub(bdiff[:], rb_sb[:, 0:1], rb_sb[:, 1:2])
            ones_sb = small.tile([1, 128], BF16, tag="ones")
            nc.vector.memset(ones_sb[:], 1.0)
            mask_row = small.tile([1, T], BF16, tag="mrow")
            mask_bc = small.tile([128, T], BF16, tag="mbc")

            prs = [
                psg.tile([1, 512], F32, tag="g", name=f"pr{th}") for th in range(TH)
            ]
            for ko in range(KD):
                xf_t = xf.tile([128, T], F32, tag="xf")
                nc.sync.dma_start(xf_t[:], xt32[ko * 128:(ko + 1) * 128, :])
                for th in range(TH):
                    nc.tensor.matmul(
                        prs[th][:], wdiff[:, ko, :], xf_t[:, ts(th, 512)],
                        start=(ko == 0), stop=(ko == KD - 1),
                    )
            # mask0 = ((l0-l1) + (b0-b1)) >= 0, as 1.0/0.0
            for th in range(TH):
                nc.vector.tensor_scalar(
                    mask_row[:, ts(th, 512)], prs[th][:], bdiff[:], 0.0,
                    mybir.AluOpType.add, mybir.AluOpType.is_ge,
                )
            # broadcast mask row across 128 partitions via K=1 matmul with ones
            for th in range(TH):
                pm = psu.tile([128, 512], F32, tag="u")
                nc.tensor.matmul(
                    pm[:], ones_sb[:], mask_row[:, ts(th, 512)], start=True, stop=True
                )
                nc.vector.tensor_copy(mask_bc[:, ts(th, 512)], pm[:])
            for ko in range(KD):
                nc.vector.tensor_mul(x0_sb[:, ko, :], xtb_sb[:, ko, :], mask_bc[:])
                nc.vector.tensor_sub(x1_sb[:, ko, :], xtb_sb[:, ko, :], x0_sb[:, ko, :])

            # ---- 3 SwiGLU paths ----
            yt_r = yt.rearrange("(md p) t -> p md t", p=128)
            yaccs = [
                yac.tile([128, T], F32, tag=f"yacc{md}", name=f"yacc{md}")
                for md in range(MD)
            ]
            xs_by_path = [xtb_sb, x0_sb, x1_sb]
            for p in range(3):
                xp = xs_by_path[p]
                hch = []
                for mf in range(MF):
                    wg_t = wgp.tile([128, KD, 128], BF16, tag="wg")
                    nc.sync.dma_start(wg_t[:], wgl[p * MF + mf])
                    wu_t = wup.tile([128, KD, 128], BF16, tag="wu")
                    nc.sync.dma_start(wu_t[:], wul[p * MF + mf])
                    h_t = hb.tile([128, T], BF16, tag="h")
                    pgs = [
                        psg.tile([128, 512], F32, tag="g", name=f"pg{th}")
                        for th in range(TH)
                    ]
                    pus = [
                        psu.tile([128, 512], F32, tag="u", name=f"pu{th}")
                        for th in range(TH)
                    ]
                    for th in range(TH):
                        for ko in range(KD):
                            nc.tensor.matmul(
                                pgs[th][:], wg_t[:, ko, :], xp[:, ko, ts(th, 512)],
                                start=(ko == 0), stop=(ko == KD - 1),
                            )
                        for ko in range(KD):
                            nc.tensor.matmul(
                                pus[th][:], wu_t[:, ko, :], xp[:, ko, ts(th, 512)],
                                start=(ko == 0), stop=(ko == KD - 1),
                            )
                    for th in range(TH):
                        pg, pu = pgs[th], pus[th]
                        g_s = gsp.tile([128, 512], BF16, tag="gs")
                        if use_silu:
                            nc.scalar.activation(
                                g_s[:], pg[:], mybir.ActivationFunctionType.Silu
                            )
                        else:
                            # CoreSim lacks Silu; g*sigmoid(g) is identical math
                            nc.scalar.activation(
                                g_s[:], pg[:], mybir.ActivationFunctionType.Sigmoid
                            )
                            nc.vector.tensor_mul(g_s[:], g_s[:], pg[:])
                        nc.vector.tensor_mul(h_t[:, ts(th, 512)], g_s[:], pu[:])
                    hch.append(h_t)
                for md in range(MD):
                    wd_t = wdp.tile([128, KF, 128], BF16, tag="wd")
                    nc.sync.dma_start(wd_t[:], wdl[p * MD + md])
                    pys = [
                        psy.tile([128, 512], F32, tag="y", name=f"py{th}")
                        for th in range(TH)
                    ]
                    for kf in range(KF):
                        for th in range(TH):
                            nc.tensor.matmul(
                                pys[th][:], wd_t[:, kf, :], hch[kf][:, ts(th, 512)],
                                start=(kf == 0), stop=(kf == KF - 1),
                            )
                    for th in range(TH):
                        if p == 0:
                            nc.vector.tensor_copy(
                                yaccs[md][:, ts(th, 512)], pys[th][:]
                            )
                        else:
                            nc.vector.tensor_add(
                                yaccs[md][:, ts(th, 512)],
                                yaccs[md][:, ts(th, 512)],
                                pys[th][:],
                            )
                    if p == 2:
                        # final path: this md slice is complete, ship it out
                        nc.sync.dma_start(yt_r[:, md, :], yaccs[md][:])
    return nc


WINDOW = 64


def build_v4(nc, T=T, D=D, F=F, W=WINDOW, use_silu=True,
             psg_b=3, psu_b=3, psy_b=2, w_b=4, hb_extra=1):
    """Host-routed variant: the host computes the router, globally sorts
    tokens by expert, and hands each core pre-sorted x^T with the expert
    boundary pinned to column T/2 +- W/2. The device runs expert0 on
    [0, T/2), expert1 on [T/2, T) unmasked, and fixes the straddle span
    with a signed-mask correction window of W tokens (masks host-supplied).
    No on-device router / sort metadata / gather; single accumulated output.
    """
    KD = D // 128   # k-tiles over D
    MF = F // 128
    MD = D // 128
    KF = F // 128
    TH = T // 512   # 512-token blocks (shared path free dim)
    half = T // 2
    HF = half       # expert block free dim (= 512, one psum bank)
    Wh = W // 2
    w0 = half - Wh

    xs = nc.dram_tensor("xs", [D, T], BF16, kind="ExternalInput").ap()
    sig = nc.dram_tensor("sig", [128, W], BF16, kind="ExternalInput").ap()
    sgn = nc.dram_tensor("sgn", [128, W], BF16, kind="ExternalInput").ap()
    wgl = nc.dram_tensor("wgl", [3 * MF, 128, KD, 128], BF16, kind="ExternalInput").ap()
    wul = nc.dram_tensor("wul", [3 * MF, 128, KD, 128], BF16, kind="ExternalInput").ap()
    wdl = nc.dram_tensor("wdl", [3 * MD, 128, KF, 128], BF16, kind="ExternalInput").ap()
    yt = nc.dram_tensor("yt", [D, T], F32, kind="ExternalOutput").ap()

    AF = mybir.ActivationFunctionType

    with tile.TileContext(nc) as tc:
        with (
            tc.tile_pool(name="xres", bufs=1) as xres,
            tc.tile_pool(name="small", bufs=1) as small,
            tc.tile_pool(name="wg", bufs=w_b) as wgp,
            tc.tile_pool(name="wu", bufs=w_b) as wup,
            tc.tile_pool(name="wd", bufs=w_b) as wdp,
            tc.tile_pool(name="hb", bufs=KF + hb_extra) as hb,
            tc.tile_pool(name="hh", bufs=KF + 1) as hhp,
            tc.tile_pool(name="hw", bufs=KF + 1) as hwp,
            tc.tile_pool(name="gs", bufs=3) as gsp,
            tc.tile_pool(name="yac", bufs=1) as yac,
            tc.tile_pool(name="psg", bufs=psg_b, space="PSUM") as psg,
            tc.tile_pool(name="psu", bufs=psu_b, space="PSUM") as psu,
            tc.tile_pool(name="psy", bufs=psy_b, space="PSUM") as psy,
        ):
            # mf=0 weight tiles lead the DMA queue so the first Ldweights
            # isn't stuck behind the 2MB xs transfer
            wg_t0 = wgp.tile([128, KD, 128], BF16, tag="wg", name="wg_t0")
            nc.sync.dma_start(wg_t0[:], wgl[0])
            wu_t0 = wup.tile([128, KD, 128], BF16, tag="wu", name="wu_t0")
            nc.sync.dma_start(wu_t0[:], wul[0])
            # sorted x^T, resident; th=0 halves first so mf=0 can start early
            xs_sb = xres.tile([128, KD, T], BF16, tag="xs")
            xs_r = xs.rearrange("(ko p) t -> p ko t", p=128)
            for th in range(TH):
                for ko in range(KD):
                    nc.sync.dma_start(
                        xs_sb[:, ko, ts(th, 512)], xs_r[:, ko, ts(th, 512)]
                    )
            sig_sb = small.tile([128, W], BF16, tag="sig")
            nc.sync.dma_start(sig_sb[:], sig)
            sgn_sb = small.tile([128, W], BF16, tag="sgn")
            nc.sync.dma_start(sgn_sb[:], sgn)

            def silu_into(psrc, wdt):
                g_s = gsp.tile([128, wdt], BF16, tag="gs", name="g_s")
                if use_silu:
                    nc.scalar.activation(g_s[:], psrc[:], AF.Silu)
                else:
                    nc.scalar.activation(g_s[:], psrc[:], AF.Sigmoid)
                    nc.vector.tensor_mul(g_s[:], g_s[:], psrc[:])
                return g_s

            yt_r = yt.rearrange("(md p) t -> p md t", p=128)
            yaccs = [
                yac.tile([128, T], F32, tag=f"yacc{md}", name=f"yacc{md}")
                for md in range(MD)
            ]

            # ---- shared path over all (sorted) tokens ----
            hch = []
            for mf in range(MF):
                if mf == 0:
                    wg_t, wu_t = wg_t0, wu_t0
                else:
                    wg_t = wgp.tile([128, KD, 128], BF16, tag="wg")
                    nc.sync.dma_start(wg_t[:], wgl[mf])
                    wu_t = wup.tile([128, KD, 128], BF16, tag="wu")
                    nc.sync.dma_start(wu_t[:], wul[mf])
                h_t = hb.tile([128, T], BF16, tag="h")
                for th in range(TH):
                    pg = psg.tile([128, 512], F32, tag="g")
                    pu = psu.tile([128, 512], F32, tag="u")
                    for ko in range(KD):
                        nc.tensor.matmul(
                            pg[:], wg_t[:, ko, :], xs_sb[:, ko, ts(th, 512)],
                            start=(ko == 0), stop=(ko == KD - 1),
                        )
                    for ko in range(KD):
                        nc.tensor.matmul(
                            pu[:], wu_t[:, ko, :], xs_sb[:, ko, ts(th, 512)],
                            start=(ko == 0), stop=(ko == KD - 1),
                        )
                    g_s = silu_into(pg, 512)
                    nc.vector.tensor_mul(h_t[:, ts(th, 512)], g_s[:], pu[:])
                hch.append(h_t)
            for md in range(MD):
                wd_t = wdp.tile([128, KF, 128], BF16, tag="wd")
                nc.sync.dma_start(wd_t[:], wdl[md])
                for th in range(TH):
                    py = psy.tile([128, 512], F32, tag="y")
                    for kf in range(KF):
                        nc.tensor.matmul(
                            py[:], wd_t[:, kf, :], hch[kf][:, ts(th, 512)],
                            start=(kf == 0), stop=(kf == KF - 1),
                        )
                    nc.vector.tensor_copy(yaccs[md][:, ts(th, 512)], py[:])

            # ---- expert blocks + correction window ----
            for e in (1, 2):
                off = 0 if e == 1 else half
                wmask = sig_sb if e == 1 else sgn_sb
                # correction window: the in-block half of this expert's
                # window is already in hA; only the out-of-block half (Wh
                # cols) needs fresh matmuls.
                oo = half if e == 1 else w0      # out-of-block global cols
                ob = (Wh, W) if e == 1 else (0, Wh)   # pos within window
                ib = (0, Wh) if e == 1 else (Wh, W)
                ib_lo = w0 if e == 1 else 0           # block-local offset
                hA = []
                hW = []
                for mf in range(MF):
                    wg_t = wgp.tile([128, KD, 128], BF16, tag="wg")
                    nc.sync.dma_start(wg_t[:], wgl[e * MF + mf])
                    wu_t = wup.tile([128, KD, 128], BF16, tag="wu")
                    nc.sync.dma_start(wu_t[:], wul[e * MF + mf])
                    hA_t = hhp.tile([128, half], BF16, tag="hh")
                    pg = psg.tile([128, HF], F32, tag="g")
                    pu = psu.tile([128, HF], F32, tag="u")
                    for ko in range(KD):
                        nc.tensor.matmul(
                            pg[:], wg_t[:, ko, :], xs_sb[:, ko, off:off + HF],
                            start=(ko == 0), stop=(ko == KD - 1),
                        )
                    for ko in range(KD):
                        nc.tensor.matmul(
                            pu[:], wu_t[:, ko, :], xs_sb[:, ko, off:off + HF],
                            start=(ko == 0), stop=(ko == KD - 1),
                        )
                    g_s = silu_into(pg, HF)
                    nc.vector.tensor_mul(hA_t[:], g_s[:], pu[:])
                    hW_t = hwp.tile([128, W], BF16, tag="hw")
                    pgw = psg.tile([128, Wh], F32, tag="g", name="pgw")
                    puw = psu.tile([128, Wh], F32, tag="u", name="puw")
                    for ko in range(KD):
                        nc.tensor.matmul(
                            pgw[:], wg_t[:, ko, :], xs_sb[:, ko, oo:oo + Wh],
                            start=(ko == 0), stop=(ko == KD - 1),
                        )
                    for ko in range(KD):
                        nc.tensor.matmul(
                            puw[:], wu_t[:, ko, :], xs_sb[:, ko, oo:oo + Wh],
                            start=(ko == 0), stop=(ko == KD - 1),
                        )
                    g_s = silu_into(pgw, Wh)
                    nc.vector.tensor_mul(hW_t[:, ob[0]:ob[1]], g_s[:], puw[:])
                    nc.vector.tensor_mul(
                        hW_t[:, ob[0]:ob[1]], hW_t[:, ob[0]:ob[1]],
                        wmask[:, ob[0]:ob[1]],
                    )
                    nc.vector.tensor_mul(
                        hW_t[:, ib[0]:ib[1]], hA_t[:, ib_lo:ib_lo + Wh],
                        wmask[:, ib[0]:ib[1]],
                    )
                    hA.append(hA_t)
                    hW.append(hW_t)
                for md in range(MD):
                    wd_t = wdp.tile([128, KF, 128], BF16, tag="wd")
                    nc.sync.dma_start(wd_t[:], wdl[e * MD + md])
                    py = psy.tile([128, HF], F32, tag="y")
                    for kf in range(KF):
                        nc.tensor.matmul(
                            py[:], wd_t[:, kf, :], hA[kf][:],
                            start=(kf == 0), stop=(kf == KF - 1),
                        )
                    nc.vector.tensor_add(
                        yaccs[md][:, off:off + HF],
                        yaccs[md][:, off:off + HF], py[:],
                    )
                    pyw = psy.tile([128, W], F32, tag="y", name="pyw")
                    for kf in range(KF):
                        nc.tensor.matmul(
                            pyw[:], wd_t[:, kf, :], hW[kf][:],
                            start=(kf == 0), stop=(kf == KF - 1),
                        )
                    nc.vector.tensor_add(
                        yaccs[md][:, w0:w0 + W],
                        yaccs[md][:, w0:w0 + W], pyw[:],
                    )
                    if e == 2:
                        nc.sync.dma_start(yt_r[:, md, :], yaccs[md][:])
    return nc


def _pack_weights(W_router, router_bias, Wg, Wu, Wd, Sg, Su, Sd):
    KD, MF, MD, KF = D // 128, F // 128, D // 128, F // 128
    G = np.stack([np.asarray(Sg), np.asarray(Wg)[0], np.asarray(Wg)[1]]).astype(np.float32)
    U = np.stack([np.asarray(Su), np.asarray(Wu)[0], np.asarray(Wu)[1]]).astype(np.float32)
    Dn = np.stack([np.asarray(Sd), np.asarray(Wd)[0], np.asarray(Wd)[1]]).astype(np.float32)
    wgl = np.ascontiguousarray(
        G.reshape(3, KD, 128, MF, 128).transpose(0, 3, 2, 1, 4)
    ).reshape(3 * MF, 128, KD, 128).astype(ml_dtypes.bfloat16)
    wul = np.ascontiguousarray(
        U.reshape(3, KD, 128, MF, 128).transpose(0, 3, 2, 1, 4)
    ).reshape(3 * MF, 128, KD, 128).astype(ml_dtypes.bfloat16)
    wdl = np.ascontiguousarray(
        Dn.reshape(3, KF, 128, MD, 128).transpose(0, 3, 2, 1, 4)
    ).reshape(3 * MD, 128, KF, 128).astype(ml_dtypes.bfloat16)
    wr_h = np.ascontiguousarray(
        np.asarray(W_router, np.float32).reshape(KD, 128, 2).transpose(1, 0, 2)
    )
    rb_h = np.asarray(router_bias, np.float32).reshape(1, 2)
    return wgl, wul, wdl, wr_h, rb_h


def pack_inputs(x, W_router, router_bias, Wg, Wu, Wd, Sg, Su, Sd, T=T, D=D, F=F):
    """Host-side sharding + layout prep for the dense fallback kernel."""
    wgl, wul, wdl, wr_h, rb_h = _pack_weights(
        W_router, router_bias, Wg, Wu, Wd, Sg, Su, Sd
    )
    flat = np.asarray(x, np.float32).reshape(-1, D)
    n_tokens = flat.shape[0]
    assert n_tokens == N_CORES * T
    xt = np.ascontiguousarray(flat.T)  # [D, N]
    xtb_full = xt.astype(ml_dtypes.bfloat16)

    in_maps = []
    for c in range(N_CORES):
        sl = slice(c * T, (c + 1) * T)
        in_maps.append({
            "xt32": np.ascontiguousarray(xt[:, sl]),
            "xtb": np.ascontiguousarray(xtb_full[:, sl]),
            "wr": wr_h,
            "rb": rb_h,
            "wgl": wgl,
            "wul": wul,
            "wdl": wdl,
        })
    return in_maps


def pack_inputs_v4(x, W_router, router_bias, Wg, Wu, Wd, Sg, Su, Sd,
                   T=T, D=D, F=F, W=WINDOW):
    """Host router + global token sort. Returns (in_maps, perms) or None if
    some core's expert split falls outside the static correction window
    (|N0 - N/2| > ~8*(W/2) - 8, a >5-sigma event) -- caller falls back.
    """
    half, Wh = T // 2, W // 2
    w0 = half - Wh
    wgl, wul, wdl, _, _ = _pack_weights(
        W_router, router_bias, Wg, Wu, Wd, Sg, Su, Sd
    )
    flat = np.asarray(x, np.float32).reshape(-1, D)
    n_tokens = flat.shape[0]
    assert n_tokens == N_CORES * T
    logits = flat @ np.asarray(W_router, np.float32)
    logits = logits + np.asarray(router_bias, np.float32)[None, :]
    to_e1 = logits[:, 1] > logits[:, 0]  # ties -> expert 0, like jnp.argmax
    idx0 = np.nonzero(~to_e1)[0]
    idx1 = np.nonzero(to_e1)[0]
    n0 = idx0.size
    base, rem = divmod(n0, N_CORES)
    counts0 = [base + (1 if c < rem else 0) for c in range(N_CORES)]
    if any(not (w0 <= k0 <= half + Wh) for k0 in counts0):
        return None
    in_maps, perms = [], []
    o0 = o1 = 0
    for c in range(N_CORES):
        k0 = counts0[c]
        k1 = T - k0
        perm = np.concatenate([idx0[o0:o0 + k0], idx1[o1:o1 + k1]])
        o0 += k0
        o1 += k1
        xs_c = np.ascontiguousarray(
            flat[perm].T.astype(ml_dtypes.bfloat16)
        )
        # e0-coefficient signed mask over window cols [w0, w0+W):
        # +1 on [half, k0) (e0 tokens computed by block B), -1 on [k0, half)
        # (e1 tokens computed by block A); e1 coefficient is the negation.
        sig = np.zeros((1, W), np.float32)
        if k0 < half:
            sig[0, k0 - w0:half - w0] = -1.0
        elif k0 > half:
            sig[0, half - w0:k0 - w0] = 1.0
        sig_bc = np.ascontiguousarray(
            np.broadcast_to(sig, (128, W)).astype(ml_dtypes.bfloat16)
        )
        sgn_bc = np.ascontiguousarray((-sig_bc).astype(ml_dtypes.bfloat16))
        in_maps.append({
            "xs": xs_c,
            "sig": sig_bc,
            "sgn": sgn_bc,
            "wgl": wgl,
            "wul": wul,
            "wdl": wdl,
        })
        perms.append(perm)
    return in_maps, perms


_CACHE = {}


def _get_compiled(ver="v4"):
    key = f"nc_{ver}"
    if key not in _CACHE:
        nc = bacc.Bacc(
            "TRN2",
            target_bir_lowering=False,
            # axon clients cannot host a BassDebugger; native path can
            debug=not axon_active(),
            num_devices=N_CORES,
        )
        if ver == "v4":
            build_v4(nc, W=WINDOW)
        else:
            build(nc)
        nc.compile()
        _CACHE[key] = nc
    return _CACHE[key]


def _run_v1(np_args, x_shape, _trace=False):
    nc = _get_compiled("v1")
    in_maps = pack_inputs(*np_args)
    res = run_bass_kernel_spmd(
        nc, in_maps, core_ids=list(range(N_CORES)), trace=_trace
    )
    out_t = np.concatenate(
        [res.results[c]["yt"] for c in range(N_CORES)], axis=1
    )
    if _trace:
        _CACHE["last_result"] = res
    return np.ascontiguousarray(out_t.T).reshape(x_shape).astype(np.float32)


def kernel(x, W_router, router_bias, Wg, Wu, Wd, Sg, Su, Sd, _trace=False, **_kw):
    np_args = (x, W_router, router_bias, Wg, Wu, Wd, Sg, Su, Sd)
    x_shape = np.asarray(x).shape
    packed = pack_inputs_v4(*np_args)
    if packed is None:
        # expert split fell outside the static correction window
        # (>5-sigma event for these inputs): run the dense kernel
        return _run_v1(np_args, x_shape, _trace)
    in_maps, perms = packed
    nc = _get_compiled("v4")
    res = run_bass_kernel_spmd(
        nc, in_maps, core_ids=list(range(N_CORES)), trace=_trace
    )
    out = np.empty((N_CORES * T, D), np.float32)
    for c in range(N_CORES):
        # yt columns are in sorted-token order; scatter back
        out[perms[c]] = res.results[c]["yt"].T
    if _trace:
        _CACHE["last_result"] = res
    return out.reshape(x_shape)


# revision 8
# speedup vs baseline: 1.1397x; 1.0159x over previous
import os
import sys

for _p in ("/opt/trn_rl_repo", "/root/.axon_site/_ro/trn_rl_repo"):
    if os.path.isdir(_p) and _p not in sys.path:
        sys.path.insert(0, _p)

import numpy as np
import ml_dtypes

import concourse.bass as bass
import concourse.tile as tile
import concourse.mybir as mybir
from concourse import bacc
from concourse._compat import axon_active
from concourse.bass import ts
from concourse.bass_utils import run_bass_kernel_spmd

N_CORES = 8
D = 1024
F = 2048
T = 1024  # tokens per core (8192 / 8)

BF16 = mybir.dt.bfloat16
F32 = mybir.dt.float32


def build(nc, T=T, D=D, F=F, use_silu=True, psg_b=3, psu_b=3, psy_b=2,
          w_b=4, hb_extra=6, xf_b=3):
    """Dense fallback: per-core MoE FFN with on-device router + masked paths.

    Layout: activations transposed (feature on partitions, tokens on free dim).
    Paths: [shared, expert0, expert1]; expert token masks folded into the
    input (x0 = x*m0, x1 = x - x0) so all three paths sum directly.
    """
    KD = D // 128   # k-tiles over D (gate/up contraction, also out tiles of down)
    MF = F // 128   # m-tiles over F
    MD = D // 128
    KF = F // 128
    TH = T // 512   # 512-token free-dim blocks

    xt32 = nc.dram_tensor("xt32", [D, T], F32, kind="ExternalInput").ap()
    xtb = nc.dram_tensor("xtb", [D, T], BF16, kind="ExternalInput").ap()
    wr = nc.dram_tensor("wr", [128, KD, 2], F32, kind="ExternalInput").ap()
    rb = nc.dram_tensor("rb", [1, 2], F32, kind="ExternalInput").ap()
    wgl = nc.dram_tensor("wgl", [3 * MF, 128, KD, 128], BF16, kind="ExternalInput").ap()
    wul = nc.dram_tensor("wul", [3 * MF, 128, KD, 128], BF16, kind="ExternalInput").ap()
    wdl = nc.dram_tensor("wdl", [3 * MD, 128, KF, 128], BF16, kind="ExternalInput").ap()
    yt = nc.dram_tensor("yt", [D, T], F32, kind="ExternalOutput").ap()

    with tile.TileContext(nc) as tc:
        with (
            tc.tile_pool(name="xres", bufs=1) as xres,
            tc.tile_pool(name="xf", bufs=xf_b) as xf,
            tc.tile_pool(name="small", bufs=1) as small,
            tc.tile_pool(name="wg", bufs=w_b) as wgp,
            tc.tile_pool(name="wu", bufs=w_b) as wup,
            tc.tile_pool(name="wd", bufs=w_b) as wdp,
            tc.tile_pool(name="hb", bufs=KF + hb_extra) as hb,
            tc.tile_pool(name="gs", bufs=3) as gsp,
            tc.tile_pool(name="yac", bufs=1) as yac,
            tc.tile_pool(name="psg", bufs=psg_b, space="PSUM") as psg,
            tc.tile_pool(name="psu", bufs=psu_b, space="PSUM") as psu,
            tc.tile_pool(name="psy", bufs=psy_b, space="PSUM") as psy,
        ):
            # resident transposed input (bf16) + masked variants
            xtb_sb = xres.tile([128, KD, T], BF16, tag="xtb")
            xtb_r = xtb.rearrange("(ko p) t -> p ko t", p=128)
            for ko in range(KD):
                nc.sync.dma_start(xtb_sb[:, ko, :], xtb_r[:, ko, :])
            x0_sb = xres.tile([128, KD, T], BF16, tag="x0")
            x1_sb = xres.tile([128, KD, T], BF16, tag="x1")

            # ---- router (fp32) ----
            wr_sb = small.tile([128, KD, 2], F32, tag="wr")
            nc.sync.dma_start(wr_sb[:], wr)
            wdiff = small.tile([128, KD, 1], F32, tag="wdiff")
            nc.vector.tensor_sub(wdiff[:], wr_sb[:, :, 0:1], wr_sb[:, :, 1:2])
            rb_sb = small.tile([1, 2], F32, tag="rb")
            nc.sync.dma_start(rb_sb[:], rb)
            bdiff = small.tile([1, 1], F32, tag="bdiff")
            nc.vector.tensor_sub(bdiff[:], rb_sb[:, 0:1], rb_sb[:, 1:2])
            ones_sb = small.tile([1, 128], BF16, tag="ones")
            nc.vector.memset(ones_sb[:], 1.0)
            mask_row = small.tile([1, T], BF16, tag="mrow")
            mask_bc = small.tile([128, T], BF16, tag="mbc")

            prs = [
                psg.tile([1, 512], F32, tag="g", name=f"pr{th}") for th in range(TH)
            ]
            for ko in range(KD):
                xf_t = xf.tile([128, T], F32, tag="xf")
                nc.sync.dma_start(xf_t[:], xt32[ko * 128:(ko + 1) * 128, :])
                for th in range(TH):
                    nc.tensor.matmul(
                        prs[th][:], wdiff[:, ko, :], xf_t[:, ts(th, 512)],
                        start=(ko == 0), stop=(ko == KD - 1),
                    )
            # mask0 = ((l0-l1) + (b0-b1)) >= 0, as 1.0/0.0
            for th in range(TH):
                nc.vector.tensor_scalar(
                    mask_row[:, ts(th, 512)], prs[th][:], bdiff[:], 0.0,
                    mybir.AluOpType.add, mybir.AluOpType.is_ge,
                )
            # broadcast mask row across 128 partitions via K=1 matmul with ones
            for th in range(TH):
                pm = psu.tile([128, 512], F32, tag="u")
                nc.tensor.matmul(
                    pm[:], ones_sb[:], mask_row[:, ts(th, 512)], start=True, stop=True
                )
                nc.vector.tensor_copy(mask_bc[:, ts(th, 512)], pm[:])
            for ko in range(KD):
                nc.vector.tensor_mul(x0_sb[:, ko, :], xtb_sb[:, ko, :], mask_bc[:])
                nc.vector.tensor_sub(x1_sb[:, ko, :], xtb_sb[:, ko, :], x0_sb[:, ko, :])

            # ---- 3 SwiGLU paths ----
            yt_r = yt.rearrange("(md p) t -> p md t", p=128)
            yaccs = [
                yac.tile([128, T], F32, tag=f"yacc{md}", name=f"yacc{md}")
                for md in range(MD)
            ]
            xs_by_path = [xtb_sb, x0_sb, x1_sb]
            for p in range(3):
                xp = xs_by_path[p]
                hch = []
                for mf in range(MF):
                    wg_t = wgp.tile([128, KD, 128], BF16, tag="wg")
                    nc.sync.dma_start(wg_t[:], wgl[p * MF + mf])
                    wu_t = wup.tile([128, KD, 128], BF16, tag="wu")
                    nc.sync.dma_start(wu_t[:], wul[p * MF + mf])
                    h_t = hb.tile([128, T], BF16, tag="h")
                    pgs = [
                        psg.tile([128, 512], F32, tag="g", name=f"pg{th}")
                        for th in range(TH)
                    ]
                    pus = [
                        psu.tile([128, 512], F32, tag="u", name=f"pu{th}")
                        for th in range(TH)
                    ]
                    for th in range(TH):
                        for ko in range(KD):
                            nc.tensor.matmul(
                                pgs[th][:], wg_t[:, ko, :], xp[:, ko, ts(th, 512)],
                                start=(ko == 0), stop=(ko == KD - 1),
                            )
                        for ko in range(KD):
                            nc.tensor.matmul(
                                pus[th][:], wu_t[:, ko, :], xp[:, ko, ts(th, 512)],
                                start=(ko == 0), stop=(ko == KD - 1),
                            )
                    for th in range(TH):
                        pg, pu = pgs[th], pus[th]
                        g_s = gsp.tile([128, 512], BF16, tag="gs")
                        if use_silu:
                            nc.scalar.activation(
                                g_s[:], pg[:], mybir.ActivationFunctionType.Silu
                            )
                        else:
                            # CoreSim lacks Silu; g*sigmoid(g) is identical math
                            nc.scalar.activation(
                                g_s[:], pg[:], mybir.ActivationFunctionType.Sigmoid
                            )
                            nc.vector.tensor_mul(g_s[:], g_s[:], pg[:])
                        nc.vector.tensor_mul(h_t[:, ts(th, 512)], g_s[:], pu[:])
                    hch.append(h_t)
                for md in range(MD):
                    wd_t = wdp.tile([128, KF, 128], BF16, tag="wd")
                    nc.sync.dma_start(wd_t[:], wdl[p * MD + md])
                    pys = [
                        psy.tile([128, 512], F32, tag="y", name=f"py{th}")
                        for th in range(TH)
                    ]
                    for kf in range(KF):
                        for th in range(TH):
                            nc.tensor.matmul(
                                pys[th][:], wd_t[:, kf, :], hch[kf][:, ts(th, 512)],
                                start=(kf == 0), stop=(kf == KF - 1),
                            )
                    for th in range(TH):
                        if p == 0:
                            nc.vector.tensor_copy(
                                yaccs[md][:, ts(th, 512)], pys[th][:]
                            )
                        else:
                            nc.vector.tensor_add(
                                yaccs[md][:, ts(th, 512)],
                                yaccs[md][:, ts(th, 512)],
                                pys[th][:],
                            )
                    if p == 2:
                        # final path: this md slice is complete, ship it out
                        nc.sync.dma_start(yt_r[:, md, :], yaccs[md][:])
    return nc


WINDOW = 64


def build_v4(nc, T=T, D=D, F=F, W=WINDOW, use_silu=True,
             psg_b=2, psu_b=2, psy_b=4, w_b=4, hb_extra=1):
    """Host-routed variant: the host computes the router, globally sorts
    tokens by expert, and hands each core pre-sorted x^T with the expert
    boundary pinned to column T/2 +- W/2. The device runs expert0 on
    [0, T/2), expert1 on [T/2, T) unmasked, and fixes the straddle span
    with a signed-mask correction window of W tokens (masks host-supplied).
    No on-device router / sort metadata / gather; single accumulated output.
    """
    KD = D // 128   # k-tiles over D
    MF = F // 128
    MD = D // 128
    KF = F // 128
    TH = T // 512   # 512-token blocks (shared path free dim)
    half = T // 2
    HF = half       # expert block free dim (= 512, one psum bank)
    Wh = W // 2
    w0 = half - Wh

    xs = nc.dram_tensor("xs", [D, T], BF16, kind="ExternalInput").ap()
    sig = nc.dram_tensor("sig", [128, W], BF16, kind="ExternalInput").ap()
    sgn = nc.dram_tensor("sgn", [128, W], BF16, kind="ExternalInput").ap()
    wgl = nc.dram_tensor("wgl", [3 * MF, 128, KD, 128], BF16, kind="ExternalInput").ap()
    wul = nc.dram_tensor("wul", [3 * MF, 128, KD, 128], BF16, kind="ExternalInput").ap()
    wdl = nc.dram_tensor("wdl", [3 * MD, 128, KF, 128], BF16, kind="ExternalInput").ap()
    yt = nc.dram_tensor("yt", [D, T], F32, kind="ExternalOutput").ap()

    AF = mybir.ActivationFunctionType

    with tile.TileContext(nc) as tc:
        with (
            tc.tile_pool(name="xres", bufs=1) as xres,
            tc.tile_pool(name="small", bufs=1) as small,
            tc.tile_pool(name="wg", bufs=w_b) as wgp,
            tc.tile_pool(name="wu", bufs=w_b) as wup,
            tc.tile_pool(name="wd", bufs=w_b) as wdp,
            tc.tile_pool(name="hb", bufs=KF + hb_extra) as hb,
            tc.tile_pool(name="hh", bufs=KF + 1) as hhp,
            tc.tile_pool(name="hw", bufs=KF + 1) as hwp,
            tc.tile_pool(name="gs", bufs=3) as gsp,
            tc.tile_pool(name="yac", bufs=1) as yac,
            tc.tile_pool(name="psg", bufs=psg_b, space="PSUM") as psg,
            tc.tile_pool(name="psu", bufs=psu_b, space="PSUM") as psu,
            tc.tile_pool(name="psy", bufs=psy_b, space="PSUM") as psy,
        ):
            # Two DMA rings: weights stream on the Pool/SWDGE ring (gpsimd),
            # activations on the SP/HWDGE ring (sync); their descriptor preps
            # run in parallel so neither stream stalls the other at startup.
            # xs strips alternate rings (th0 first) to pipeline arrival.
            wg_t0 = wgp.tile([128, KD, 128], BF16, tag="wg", name="wg_t0")
            nc.gpsimd.dma_start(wg_t0[:], wgl[0])
            wu_t0 = wup.tile([128, KD, 128], BF16, tag="wu", name="wu_t0")
            nc.gpsimd.dma_start(wu_t0[:], wul[0])
            xs_sb = xres.tile([128, KD, T], BF16, tag="xs")
            xs_r = xs.rearrange("(ko p) t -> p ko t", p=128)
            for th in range(TH):
                for ko in range(KD):
                    eng = nc.sync if ko % 2 == 0 else nc.gpsimd
                    eng.dma_start(
                        xs_sb[:, ko, ts(th, 512)], xs_r[:, ko, ts(th, 512)]
                    )
            sig_sb = small.tile([128, W], BF16, tag="sig")
            sgn_sb = small.tile([128, W], BF16, tag="sgn")

            def silu_into(psrc, wdt):
                g_s = gsp.tile([128, wdt], BF16, tag="gs", name="g_s")
                if use_silu:
                    nc.scalar.activation(g_s[:], psrc[:], AF.Silu)
                else:
                    nc.scalar.activation(g_s[:], psrc[:], AF.Sigmoid)
                    nc.vector.tensor_mul(g_s[:], g_s[:], psrc[:])
                return g_s

            yt_r = yt.rearrange("(md p) t -> p md t", p=128)
            yaccs = [
                yac.tile([128, T], F32, tag=f"yacc{md}", name=f"yacc{md}")
                for md in range(MD)
            ]

            # ---- shared path over all (sorted) tokens ----
            hch = []
            for mf in range(MF):
                if mf == 0:
                    wg_t, wu_t = wg_t0, wu_t0
                else:
                    wg_t = wgp.tile([128, KD, 128], BF16, tag="wg")
                    nc.gpsimd.dma_start(wg_t[:], wgl[mf])
                    wu_t = wup.tile([128, KD, 128], BF16, tag="wu")
                    nc.gpsimd.dma_start(wu_t[:], wul[mf])
                h_t = hb.tile([128, T], BF16, tag="h")
                for th in range(TH):
                    pg = psg.tile([128, 512], F32, tag="g")
                    pu = psu.tile([128, 512], F32, tag="u")
                    for ko in range(KD):
                        nc.tensor.matmul(
                            pg[:], wg_t[:, ko, :], xs_sb[:, ko, ts(th, 512)],
                            start=(ko == 0), stop=(ko == KD - 1),
                        )
                    for ko in range(KD):
                        nc.tensor.matmul(
                            pu[:], wu_t[:, ko, :], xs_sb[:, ko, ts(th, 512)],
                            start=(ko == 0), stop=(ko == KD - 1),
                        )
                    g_s = silu_into(pg, 512)
                    nc.vector.tensor_mul(h_t[:, ts(th, 512)], g_s[:], pu[:])
                hch.append(h_t)
            for md in range(MD):
                wd_t = wdp.tile([128, KF, 128], BF16, tag="wd")
                nc.gpsimd.dma_start(wd_t[:], wdl[md])
                for th in range(TH):
                    py = psy.tile([128, 512], F32, tag="y")
                    for kf in range(KF):
                        nc.tensor.matmul(
                            py[:], wd_t[:, kf, :], hch[kf][:, ts(th, 512)],
                            start=(kf == 0), stop=(kf == KF - 1),
                        )
                    nc.vector.tensor_copy(yaccs[md][:, ts(th, 512)], py[:])

            # window masks: needed only from here on; keep their DMAs out of
            # the startup-critical queues
            nc.gpsimd.dma_start(sig_sb[:], sig)
            nc.gpsimd.dma_start(sgn_sb[:], sgn)

            # ---- expert blocks + correction window ----
            for e in (1, 2):
                off = 0 if e == 1 else half
                wmask = sig_sb if e == 1 else sgn_sb
                # correction window: the in-block half of this expert's
                # window is already in hA; only the out-of-block half (Wh
                # cols) needs fresh matmuls.
                oo = half if e == 1 else w0      # out-of-block global cols
                ob = (Wh, W) if e == 1 else (0, Wh)   # pos within window
                ib = (0, Wh) if e == 1 else (Wh, W)
                ib_lo = w0 if e == 1 else 0           # block-local offset
                hA = []
                hW = []
                for mf in range(MF):
                    wg_t = wgp.tile([128, KD, 128], BF16, tag="wg")
                    nc.gpsimd.dma_start(wg_t[:], wgl[e * MF + mf])
                    wu_t = wup.tile([128, KD, 128], BF16, tag="wu")
                    nc.gpsimd.dma_start(wu_t[:], wul[e * MF + mf])
                    hA_t = hhp.tile([128, half], BF16, tag="hh")
                    pg = psg.tile([128, HF], F32, tag="g")
                    pu = psu.tile([128, HF], F32, tag="u")
                    for ko in range(KD):
                        nc.tensor.matmul(
                            pg[:], wg_t[:, ko, :], xs_sb[:, ko, off:off + HF],
                            start=(ko == 0), stop=(ko == KD - 1),
                        )
                    for ko in range(KD):
                        nc.tensor.matmul(
                            pu[:], wu_t[:, ko, :], xs_sb[:, ko, off:off + HF],
                            start=(ko == 0), stop=(ko == KD - 1),
                        )
                    g_s = silu_into(pg, HF)
                    nc.vector.tensor_mul(hA_t[:], g_s[:], pu[:])
                    hW_t = hwp.tile([128, W], BF16, tag="hw")
                    pgw = psg.tile([128, Wh], F32, tag="g", name="pgw")
                    puw = psu.tile([128, Wh], F32, tag="u", name="puw")
                    for ko in range(KD):
                        nc.tensor.matmul(
                            pgw[:], wg_t[:, ko, :], xs_sb[:, ko, oo:oo + Wh],
                            start=(ko == 0), stop=(ko == KD - 1),
                        )
                    for ko in range(KD):
                        nc.tensor.matmul(
                            puw[:], wu_t[:, ko, :], xs_sb[:, ko, oo:oo + Wh],
                            start=(ko == 0), stop=(ko == KD - 1),
                        )
                    g_s = silu_into(pgw, Wh)
                    nc.vector.tensor_mul(hW_t[:, ob[0]:ob[1]], g_s[:], puw[:])
                    nc.vector.tensor_mul(
                        hW_t[:, ob[0]:ob[1]], hW_t[:, ob[0]:ob[1]],
                        wmask[:, ob[0]:ob[1]],
                    )
                    nc.vector.tensor_mul(
                        hW_t[:, ib[0]:ib[1]], hA_t[:, ib_lo:ib_lo + Wh],
                        wmask[:, ib[0]:ib[1]],
                    )
                    hA.append(hA_t)
                    hW.append(hW_t)
                for md in range(MD):
                    wd_t = wdp.tile([128, KF, 128], BF16, tag="wd")
                    nc.gpsimd.dma_start(wd_t[:], wdl[e * MD + md])
                    if e == 1:
                        # block first: cols [0, w0) are final right after the
                        # block add, so their output DMA overlaps e=2 compute
                        py = psy.tile([128, HF], F32, tag="y")
                        for kf in range(KF):
                            nc.tensor.matmul(
                                py[:], wd_t[:, kf, :], hA[kf][:],
                                start=(kf == 0), stop=(kf == KF - 1),
                            )
                        nc.vector.tensor_add(
                            yaccs[md][:, off:off + HF],
                            yaccs[md][:, off:off + HF], py[:],
                        )
                        nc.sync.dma_start(yt_r[:, md, 0:w0], yaccs[md][:, 0:w0])
                        pyw = psy.tile([128, W], F32, tag="y", name="pyw")
                        for kf in range(KF):
                            nc.tensor.matmul(
                                pyw[:], wd_t[:, kf, :], hW[kf][:],
                                start=(kf == 0), stop=(kf == KF - 1),
                            )
                        nc.vector.tensor_add(
                            yaccs[md][:, w0:w0 + W],
                            yaccs[md][:, w0:w0 + W], pyw[:],
                        )
                    else:
                        # window first so the final DVE add (and the output
                        # DMA it gates) waits only on the block add
                        pyw = psy.tile([128, W], F32, tag="y", name="pyw")
                        for kf in range(KF):
                            nc.tensor.matmul(
                                pyw[:], wd_t[:, kf, :], hW[kf][:],
                                start=(kf == 0), stop=(kf == KF - 1),
                            )
                        nc.vector.tensor_add(
                            yaccs[md][:, w0:w0 + W],
                            yaccs[md][:, w0:w0 + W], pyw[:],
                        )
                        py = psy.tile([128, HF], F32, tag="y")
                        for kf in range(KF):
                            nc.tensor.matmul(
                                py[:], wd_t[:, kf, :], hA[kf][:],
                                start=(kf == 0), stop=(kf == KF - 1),
                            )
                        nc.vector.tensor_add(
                            yaccs[md][:, off:off + HF],
                            yaccs[md][:, off:off + HF], py[:],
                        )
                        nc.sync.dma_start(yt_r[:, md, w0:T], yaccs[md][:, w0:T])
    return nc


def _pack_weights(W_router, router_bias, Wg, Wu, Wd, Sg, Su, Sd):
    KD, MF, MD, KF = D // 128, F // 128, D // 128, F // 128
    G = np.stack([np.asarray(Sg), np.asarray(Wg)[0], np.asarray(Wg)[1]]).astype(np.float32)
    U = np.stack([np.asarray(Su), np.asarray(Wu)[0], np.asarray(Wu)[1]]).astype(np.float32)
    Dn = np.stack([np.asarray(Sd), np.asarray(Wd)[0], np.asarray(Wd)[1]]).astype(np.float32)
    wgl = np.ascontiguousarray(
        G.reshape(3, KD, 128, MF, 128).transpose(0, 3, 2, 1, 4)
    ).reshape(3 * MF, 128, KD, 128).astype(ml_dtypes.bfloat16)
    wul = np.ascontiguousarray(
        U.reshape(3, KD, 128, MF, 128).transpose(0, 3, 2, 1, 4)
    ).reshape(3 * MF, 128, KD, 128).astype(ml_dtypes.bfloat16)
    wdl = np.ascontiguousarray(
        Dn.reshape(3, KF, 128, MD, 128).transpose(0, 3, 2, 1, 4)
    ).reshape(3 * MD, 128, KF, 128).astype(ml_dtypes.bfloat16)
    wr_h = np.ascontiguousarray(
        np.asarray(W_router, np.float32).reshape(KD, 128, 2).transpose(1, 0, 2)
    )
    rb_h = np.asarray(router_bias, np.float32).reshape(1, 2)
    return wgl, wul, wdl, wr_h, rb_h


def pack_inputs(x, W_router, router_bias, Wg, Wu, Wd, Sg, Su, Sd, T=T, D=D, F=F):
    """Host-side sharding + layout prep for the dense fallback kernel."""
    wgl, wul, wdl, wr_h, rb_h = _pack_weights(
        W_router, router_bias, Wg, Wu, Wd, Sg, Su, Sd
    )
    flat = np.asarray(x, np.float32).reshape(-1, D)
    n_tokens = flat.shape[0]
    assert n_tokens == N_CORES * T
    xt = np.ascontiguousarray(flat.T)  # [D, N]
    xtb_full = xt.astype(ml_dtypes.bfloat16)

    in_maps = []
    for c in range(N_CORES):
        sl = slice(c * T, (c + 1) * T)
        in_maps.append({
            "xt32": np.ascontiguousarray(xt[:, sl]),
            "xtb": np.ascontiguousarray(xtb_full[:, sl]),
            "wr": wr_h,
            "rb": rb_h,
            "wgl": wgl,
            "wul": wul,
            "wdl": wdl,
        })
    return in_maps


def pack_inputs_v4(x, W_router, router_bias, Wg, Wu, Wd, Sg, Su, Sd,
                   T=T, D=D, F=F, W=WINDOW):
    """Host router + global token sort. Returns (in_maps, perms) or None if
    some core's expert split falls outside the static correction window
    (|N0 - N/2| > ~8*(W/2) - 8, a >5-sigma event) -- caller falls back.
    """
    half, Wh = T // 2, W // 2
    w0 = half - Wh
    wgl, wul, wdl, _, _ = _pack_weights(
        W_router, router_bias, Wg, Wu, Wd, Sg, Su, Sd
    )
    flat = np.asarray(x, np.float32).reshape(-1, D)
    n_tokens = flat.shape[0]
    assert n_tokens == N_CORES * T
    logits = flat @ np.asarray(W_router, np.float32)
    logits = logits + np.asarray(router_bias, np.float32)[None, :]
    to_e1 = logits[:, 1] > logits[:, 0]  # ties -> expert 0, like jnp.argmax
    idx0 = np.nonzero(~to_e1)[0]
    idx1 = np.nonzero(to_e1)[0]
    n0 = idx0.size
    base, rem = divmod(n0, N_CORES)
    counts0 = [base + (1 if c < rem else 0) for c in range(N_CORES)]
    if any(not (w0 <= k0 <= half + Wh) for k0 in counts0):
        return None
    in_maps, perms = [], []
    o0 = o1 = 0
    for c in range(N_CORES):
        k0 = counts0[c]
        k1 = T - k0
        perm = np.concatenate([idx0[o0:o0 + k0], idx1[o1:o1 + k1]])
        o0 += k0
        o1 += k1
        xs_c = np.ascontiguousarray(
            flat[perm].T.astype(ml_dtypes.bfloat16)
        )
        # e0-coefficient signed mask over window cols [w0, w0+W):
        # +1 on [half, k0) (e0 tokens computed by block B), -1 on [k0, half)
        # (e1 tokens computed by block A); e1 coefficient is the negation.
        sig = np.zeros((1, W), np.float32)
        if k0 < half:
            sig[0, k0 - w0:half - w0] = -1.0
        elif k0 > half:
            sig[0, half - w0:k0 - w0] = 1.0
        sig_bc = np.ascontiguousarray(
            np.broadcast_to(sig, (128, W)).astype(ml_dtypes.bfloat16)
        )
        sgn_bc = np.ascontiguousarray((-sig_bc).astype(ml_dtypes.bfloat16))
        in_maps.append({
            "xs": xs_c,
            "sig": sig_bc,
            "sgn": sgn_bc,
            "wgl": wgl,
            "wul": wul,
            "wdl": wdl,
        })
        perms.append(perm)
    return in_maps, perms


_CACHE = {}


def _get_compiled(ver="v4"):
    key = f"nc_{ver}"
    if key not in _CACHE:
        nc = bacc.Bacc(
            "TRN2",
            target_bir_lowering=False,
            # axon clients cannot host a BassDebugger; native path can
            debug=not axon_active(),
            num_devices=N_CORES,
        )
        if ver == "v4":
            build_v4(nc, W=WINDOW)
        else:
            build(nc)
        nc.compile()
        _CACHE[key] = nc
    return _CACHE[key]


def _run_v1(np_args, x_shape, _trace=False):
    nc = _get_compiled("v1")
    in_maps = pack_inputs(*np_args)
    res = run_bass_kernel_spmd(
        nc, in_maps, core_ids=list(range(N_CORES)), trace=_trace
    )
    out_t = np.concatenate(
        [res.results[c]["yt"] for c in range(N_CORES)], axis=1
    )
    if _trace:
        _CACHE["last_result"] = res
    return np.ascontiguousarray(out_t.T).reshape(x_shape).astype(np.float32)


def kernel(x, W_router, router_bias, Wg, Wu, Wd, Sg, Su, Sd, _trace=False, **_kw):
    np_args = (x, W_router, router_bias, Wg, Wu, Wd, Sg, Su, Sd)
    x_shape = np.asarray(x).shape
    packed = pack_inputs_v4(*np_args)
    if packed is None:
        # expert split fell outside the static correction window
        # (>5-sigma event for these inputs): run the dense kernel
        return _run_v1(np_args, x_shape, _trace)
    in_maps, perms = packed
    nc = _get_compiled("v4")
    res = run_bass_kernel_spmd(
        nc, in_maps, core_ids=list(range(N_CORES)), trace=_trace
    )
    out = np.empty((N_CORES * T, D), np.float32)
    for c in range(N_CORES):
        # yt columns are in sorted-token order; scatter back
        out[perms[c]] = res.results[c]["yt"].T
    if _trace:
        _CACHE["last_result"] = res
    return out.reshape(x_shape)


# revision 10
# speedup vs baseline: 1.1764x; 1.0322x over previous
import os
import sys

for _p in ("/opt/trn_rl_repo", "/root/.axon_site/_ro/trn_rl_repo"):
    if os.path.isdir(_p) and _p not in sys.path:
        sys.path.insert(0, _p)

import numpy as np
import ml_dtypes

import concourse.bass as bass
import concourse.tile as tile
import concourse.mybir as mybir
from concourse import bacc
from concourse._compat import axon_active
from concourse.bass import ts
from concourse.bass_utils import run_bass_kernel_spmd

N_CORES = 8
D = 1024
F = 2048
T = 1024  # tokens per core (8192 / 8)

BF16 = mybir.dt.bfloat16
F32 = mybir.dt.float32


def build(nc, T=T, D=D, F=F, use_silu=True, psg_b=3, psu_b=3, psy_b=2,
          w_b=4, hb_extra=6, xf_b=3):
    """Dense fallback: per-core MoE FFN with on-device router + masked paths.

    Layout: activations transposed (feature on partitions, tokens on free dim).
    Paths: [shared, expert0, expert1]; expert token masks folded into the
    input (x0 = x*m0, x1 = x - x0) so all three paths sum directly.
    """
    KD = D // 128   # k-tiles over D (gate/up contraction, also out tiles of down)
    MF = F // 128   # m-tiles over F
    MD = D // 128
    KF = F // 128
    TH = T // 512   # 512-token free-dim blocks

    xt32 = nc.dram_tensor("xt32", [D, T], F32, kind="ExternalInput").ap()
    xtb = nc.dram_tensor("xtb", [D, T], BF16, kind="ExternalInput").ap()
    wr = nc.dram_tensor("wr", [128, KD, 2], F32, kind="ExternalInput").ap()
    rb = nc.dram_tensor("rb", [1, 2], F32, kind="ExternalInput").ap()
    wgl = nc.dram_tensor("wgl", [3 * MF, 128, KD, 128], BF16, kind="ExternalInput").ap()
    wul = nc.dram_tensor("wul", [3 * MF, 128, KD, 128], BF16, kind="ExternalInput").ap()
    wdl = nc.dram_tensor("wdl", [3 * MD, 128, KF, 128], BF16, kind="ExternalInput").ap()
    yt = nc.dram_tensor("yt", [D, T], F32, kind="ExternalOutput").ap()

    with tile.TileContext(nc) as tc:
        with (
            tc.tile_pool(name="xres", bufs=1) as xres,
            tc.tile_pool(name="xf", bufs=xf_b) as xf,
            tc.tile_pool(name="small", bufs=1) as small,
            tc.tile_pool(name="wg", bufs=w_b) as wgp,
            tc.tile_pool(name="wu", bufs=w_b) as wup,
            tc.tile_pool(name="wd", bufs=w_b) as wdp,
            tc.tile_pool(name="hb", bufs=KF + hb_extra) as hb,
            tc.tile_pool(name="gs", bufs=3) as gsp,
            tc.tile_pool(name="yac", bufs=1) as yac,
            tc.tile_pool(name="psg", bufs=psg_b, space="PSUM") as psg,
            tc.tile_pool(name="psu", bufs=psu_b, space="PSUM") as psu,
            tc.tile_pool(name="psy", bufs=psy_b, space="PSUM") as psy,
        ):
            # resident transposed input (bf16) + masked variants
            xtb_sb = xres.tile([128, KD, T], BF16, tag="xtb")
            xtb_r = xtb.rearrange("(ko p) t -> p ko t", p=128)
            for ko in range(KD):
                nc.sync.dma_start(xtb_sb[:, ko, :], xtb_r[:, ko, :])
            x0_sb = xres.tile([128, KD, T], BF16, tag="x0")
            x1_sb = xres.tile([128, KD, T], BF16, tag="x1")

            # ---- router (fp32) ----
            wr_sb = small.tile([128, KD, 2], F32, tag="wr")
            nc.sync.dma_start(wr_sb[:], wr)
            wdiff = small.tile([128, KD, 1], F32, tag="wdiff")
            nc.vector.tensor_sub(wdiff[:], wr_sb[:, :, 0:1], wr_sb[:, :, 1:2])
            rb_sb = small.tile([1, 2], F32, tag="rb")
            nc.sync.dma_start(rb_sb[:], rb)
            bdiff = small.tile([1, 1], F32, tag="bdiff")
            nc.vector.tensor_sub(bdiff[:], rb_sb[:, 0:1], rb_sb[:, 1:2])
            ones_sb = small.tile([1, 128], BF16, tag="ones")
            nc.vector.memset(ones_sb[:], 1.0)
            mask_row = small.tile([1, T], BF16, tag="mrow")
            mask_bc = small.tile([128, T], BF16, tag="mbc")

            prs = [
                psg.tile([1, 512], F32, tag="g", name=f"pr{th}") for th in range(TH)
            ]
            for ko in range(KD):
                xf_t = xf.tile([128, T], F32, tag="xf")
                nc.sync.dma_start(xf_t[:], xt32[ko * 128:(ko + 1) * 128, :])
                for th in range(TH):
                    nc.tensor.matmul(
                        prs[th][:], wdiff[:, ko, :], xf_t[:, ts(th, 512)],
                        start=(ko == 0), stop=(ko == KD - 1),
                    )
            # mask0 = ((l0-l1) + (b0-b1)) >= 0, as 1.0/0.0
            for th in range(TH):
                nc.vector.tensor_scalar(
                    mask_row[:, ts(th, 512)], prs[th][:], bdiff[:], 0.0,
                    mybir.AluOpType.add, mybir.AluOpType.is_ge,
                )
            # broadcast mask row across 128 partitions via K=1 matmul with ones
            for th in range(TH):
                pm = psu.tile([128, 512], F32, tag="u")
                nc.tensor.matmul(
                    pm[:], ones_sb[:], mask_row[:, ts(th, 512)], start=True, stop=True
                )
                nc.vector.tensor_copy(mask_bc[:, ts(th, 512)], pm[:])
            for ko in range(KD):
                nc.vector.tensor_mul(x0_sb[:, ko, :], xtb_sb[:, ko, :], mask_bc[:])
                nc.vector.tensor_sub(x1_sb[:, ko, :], xtb_sb[:, ko, :], x0_sb[:, ko, :])

            # ---- 3 SwiGLU paths ----
            yt_r = yt.rearrange("(md p) t -> p md t", p=128)
            yaccs = [
                yac.tile([128, T], F32, tag=f"yacc{md}", name=f"yacc{md}")
                for md in range(MD)
            ]
            xs_by_path = [xtb_sb, x0_sb, x1_sb]
            for p in range(3):
                xp = xs_by_path[p]
                hch = []
                for mf in range(MF):
                    wg_t = wgp.tile([128, KD, 128], BF16, tag="wg")
                    nc.sync.dma_start(wg_t[:], wgl[p * MF + mf])
                    wu_t = wup.tile([128, KD, 128], BF16, tag="wu")
                    nc.sync.dma_start(wu_t[:], wul[p * MF + mf])
                    h_t = hb.tile([128, T], BF16, tag="h")
                    pgs = [
                        psg.tile([128, 512], F32, tag="g", name=f"pg{th}")
                        for th in range(TH)
                    ]
                    pus = [
                        psu.tile([128, 512], F32, tag="u", name=f"pu{th}")
                        for th in range(TH)
                    ]
                    for th in range(TH):
                        for ko in range(KD):
                            nc.tensor.matmul(
                                pgs[th][:], wg_t[:, ko, :], xp[:, ko, ts(th, 512)],
                                start=(ko == 0), stop=(ko == KD - 1),
                            )
                        for ko in range(KD):
                            nc.tensor.matmul(
                                pus[th][:], wu_t[:, ko, :], xp[:, ko, ts(th, 512)],
                                start=(ko == 0), stop=(ko == KD - 1),
                            )
                    for th in range(TH):
                        pg, pu = pgs[th], pus[th]
                        g_s = gsp.tile([128, 512], BF16, tag="gs")
                        if use_silu:
                            nc.scalar.activation(
                                g_s[:], pg[:], mybir.ActivationFunctionType.Silu
                            )
                        else:
                            # CoreSim lacks Silu; g*sigmoid(g) is identical math
                            nc.scalar.activation(
                                g_s[:], pg[:], mybir.ActivationFunctionType.Sigmoid
                            )
                            nc.vector.tensor_mul(g_s[:], g_s[:], pg[:])
                        nc.vector.tensor_mul(h_t[:, ts(th, 512)], g_s[:], pu[:])
                    hch.append(h_t)
                for md in range(MD):
                    wd_t = wdp.tile([128, KF, 128], BF16, tag="wd")
                    nc.sync.dma_start(wd_t[:], wdl[p * MD + md])
                    pys = [
                        psy.tile([128, 512], F32, tag="y", name=f"py{th}")
                        for th in range(TH)
                    ]
                    for kf in range(KF):
                        for th in range(TH):
                            nc.tensor.matmul(
                                pys[th][:], wd_t[:, kf, :], hch[kf][:, ts(th, 512)],
                                start=(kf == 0), stop=(kf == KF - 1),
                            )
                    for th in range(TH):
                        if p == 0:
                            nc.vector.tensor_copy(
                                yaccs[md][:, ts(th, 512)], pys[th][:]
                            )
                        else:
                            nc.vector.tensor_add(
                                yaccs[md][:, ts(th, 512)],
                                yaccs[md][:, ts(th, 512)],
                                pys[th][:],
                            )
                    if p == 2:
                        # final path: this md slice is complete, ship it out
                        nc.sync.dma_start(yt_r[:, md, :], yaccs[md][:])
    return nc


WINDOW = 16


def build_v4(nc, T=T, D=D, F=F, W=WINDOW, use_silu=True,
             psg_b=2, psu_b=2, psy_b=4, w_b=4, hb_extra=1):
    """Host-routed variant: the host computes the router, globally sorts
    tokens by expert, and hands each core pre-sorted x^T with the expert
    boundary pinned to column T/2 +- W/2. The device runs expert0 on
    [0, T/2), expert1 on [T/2, T) unmasked, and fixes the straddle span
    with a signed-mask correction window of W tokens (masks host-supplied).
    No on-device router / sort metadata / gather; single accumulated output.
    """
    KD = D // 128   # k-tiles over D
    MF = F // 128
    MD = D // 128
    KF = F // 128
    TH = T // 512   # 512-token blocks (shared path free dim)
    half = T // 2
    HF = half       # expert block free dim (= 512, one psum bank)
    Wh = W // 2
    w0 = half - Wh

    xs = nc.dram_tensor("xs", [D, T], BF16, kind="ExternalInput").ap()
    sig = nc.dram_tensor("sig", [128, W], BF16, kind="ExternalInput").ap()
    sgn = nc.dram_tensor("sgn", [128, W], BF16, kind="ExternalInput").ap()
    wgl = nc.dram_tensor("wgl", [3 * MF, 128, KD, 128], BF16, kind="ExternalInput").ap()
    wul = nc.dram_tensor("wul", [3 * MF, 128, KD, 128], BF16, kind="ExternalInput").ap()
    wdl = nc.dram_tensor("wdl", [3 * MD, 128, KF, 128], BF16, kind="ExternalInput").ap()
    yt = nc.dram_tensor("yt", [D, T], F32, kind="ExternalOutput").ap()

    AF = mybir.ActivationFunctionType

    with tile.TileContext(nc) as tc:
        with (
            tc.tile_pool(name="xres", bufs=1) as xres,
            tc.tile_pool(name="small", bufs=1) as small,
            tc.tile_pool(name="wg", bufs=w_b) as wgp,
            tc.tile_pool(name="wu", bufs=w_b) as wup,
            tc.tile_pool(name="wd", bufs=w_b) as wdp,
            tc.tile_pool(name="hb", bufs=KF + hb_extra) as hb,
            tc.tile_pool(name="hh", bufs=KF + 1) as hhp,
            tc.tile_pool(name="hw", bufs=KF + 1) as hwp,
            tc.tile_pool(name="gs", bufs=3) as gsp,
            tc.tile_pool(name="yac", bufs=1) as yac,
            tc.tile_pool(name="psg", bufs=psg_b, space="PSUM") as psg,
            tc.tile_pool(name="psu", bufs=psu_b, space="PSUM") as psu,
            tc.tile_pool(name="psy", bufs=psy_b, space="PSUM") as psy,
        ):
            # Two DMA rings: weights stream on the Pool/SWDGE ring (gpsimd),
            # activations on the SP/HWDGE ring (sync); their descriptor preps
            # run in parallel so neither stream stalls the other at startup.
            # The first ko slice of wg0 leads the sync ring so the opening
            # Ldweights fires ~1us earlier; xs strips are split across rings
            # roughly matching each ring's prep rate vs the PE demand order.
            wg_t0 = wgp.tile([128, KD, 128], BF16, tag="wg", name="wg_t0")
            nc.sync.dma_start(wg_t0[:, 0:1, :], wgl[0][:, 0:1, :])
            nc.gpsimd.dma_start(wg_t0[:, 1:KD, :], wgl[0][:, 1:KD, :])
            wu_t0 = wup.tile([128, KD, 128], BF16, tag="wu", name="wu_t0")
            nc.gpsimd.dma_start(wu_t0[:], wul[0])
            xs_sb = xres.tile([128, KD, T], BF16, tag="xs")
            xs_r = xs.rearrange("(ko p) t -> p ko t", p=128)
            sync_strips = {(0, 0), (0, 1), (0, 2), (0, 3), (0, 5),
                           (1, 0), (1, 2), (1, 4), (1, 6)}
            for th in range(TH):
                for ko in range(KD):
                    eng = nc.sync if (th, ko) in sync_strips else nc.gpsimd
                    eng.dma_start(
                        xs_sb[:, ko, ts(th, 512)], xs_r[:, ko, ts(th, 512)]
                    )
            sig_sb = small.tile([128, W], BF16, tag="sig")
            sgn_sb = small.tile([128, W], BF16, tag="sgn")

            def silu_into(psrc, wdt):
                g_s = gsp.tile([128, wdt], BF16, tag="gs", name="g_s")
                if use_silu:
                    nc.scalar.activation(g_s[:], psrc[:], AF.Silu)
                else:
                    nc.scalar.activation(g_s[:], psrc[:], AF.Sigmoid)
                    nc.vector.tensor_mul(g_s[:], g_s[:], psrc[:])
                return g_s

            yt_r = yt.rearrange("(md p) t -> p md t", p=128)
            yaccs = [
                yac.tile([128, T], F32, tag=f"yacc{md}", name=f"yacc{md}")
                for md in range(MD)
            ]

            # ---- shared path over all (sorted) tokens ----
            hch = []
            for mf in range(MF):
                if mf == 0:
                    wg_t, wu_t = wg_t0, wu_t0
                else:
                    wg_t = wgp.tile([128, KD, 128], BF16, tag="wg")
                    nc.gpsimd.dma_start(wg_t[:], wgl[mf])
                    wu_t = wup.tile([128, KD, 128], BF16, tag="wu")
                    nc.gpsimd.dma_start(wu_t[:], wul[mf])
                h_t = hb.tile([128, T], BF16, tag="h")
                for th in range(TH):
                    pg = psg.tile([128, 512], F32, tag="g")
                    pu = psu.tile([128, 512], F32, tag="u")
                    for ko in range(KD):
                        nc.tensor.matmul(
                            pg[:], wg_t[:, ko, :], xs_sb[:, ko, ts(th, 512)],
                            start=(ko == 0), stop=(ko == KD - 1),
                        )
                    for ko in range(KD):
                        nc.tensor.matmul(
                            pu[:], wu_t[:, ko, :], xs_sb[:, ko, ts(th, 512)],
                            start=(ko == 0), stop=(ko == KD - 1),
                        )
                    g_s = silu_into(pg, 512)
                    nc.vector.tensor_mul(h_t[:, ts(th, 512)], g_s[:], pu[:])
                hch.append(h_t)
            for md in range(MD):
                wd_t = wdp.tile([128, KF, 128], BF16, tag="wd")
                nc.gpsimd.dma_start(wd_t[:], wdl[md])
                for th in range(TH):
                    py = psy.tile([128, 512], F32, tag="y")
                    for kf in range(KF):
                        nc.tensor.matmul(
                            py[:], wd_t[:, kf, :], hch[kf][:, ts(th, 512)],
                            start=(kf == 0), stop=(kf == KF - 1),
                        )
                    nc.vector.tensor_copy(yaccs[md][:, ts(th, 512)], py[:])

            # window masks: needed only from here on; keep their DMAs out of
            # the startup-critical queues
            nc.gpsimd.dma_start(sig_sb[:], sig)
            nc.gpsimd.dma_start(sgn_sb[:], sgn)

            # ---- expert blocks + correction window ----
            for e in (1, 2):
                off = 0 if e == 1 else half
                wmask = sig_sb if e == 1 else sgn_sb
                # correction window: the in-block half of this expert's
                # window is already in hA; only the out-of-block half (Wh
                # cols) needs fresh matmuls.
                oo = half if e == 1 else w0      # out-of-block global cols
                ob = (Wh, W) if e == 1 else (0, Wh)   # pos within window
                ib = (0, Wh) if e == 1 else (Wh, W)
                ib_lo = w0 if e == 1 else 0           # block-local offset
                hA = []
                hW = []
                for mf in range(MF):
                    wg_t = wgp.tile([128, KD, 128], BF16, tag="wg")
                    nc.gpsimd.dma_start(wg_t[:], wgl[e * MF + mf])
                    wu_t = wup.tile([128, KD, 128], BF16, tag="wu")
                    nc.gpsimd.dma_start(wu_t[:], wul[e * MF + mf])
                    hA_t = hhp.tile([128, half], BF16, tag="hh")
                    pg = psg.tile([128, HF], F32, tag="g")
                    pu = psu.tile([128, HF], F32, tag="u")
                    for ko in range(KD):
                        nc.tensor.matmul(
                            pg[:], wg_t[:, ko, :], xs_sb[:, ko, off:off + HF],
                            start=(ko == 0), stop=(ko == KD - 1),
                        )
                    for ko in range(KD):
                        nc.tensor.matmul(
                            pu[:], wu_t[:, ko, :], xs_sb[:, ko, off:off + HF],
                            start=(ko == 0), stop=(ko == KD - 1),
                        )
                    g_s = silu_into(pg, HF)
                    nc.vector.tensor_mul(hA_t[:], g_s[:], pu[:])
                    hW_t = hwp.tile([128, W], BF16, tag="hw")
                    pgw = psg.tile([128, Wh], F32, tag="g", name="pgw")
                    puw = psu.tile([128, Wh], F32, tag="u", name="puw")
                    for ko in range(KD):
                        nc.tensor.matmul(
                            pgw[:], wg_t[:, ko, :], xs_sb[:, ko, oo:oo + Wh],
                            start=(ko == 0), stop=(ko == KD - 1),
                        )
                    for ko in range(KD):
                        nc.tensor.matmul(
                            puw[:], wu_t[:, ko, :], xs_sb[:, ko, oo:oo + Wh],
                            start=(ko == 0), stop=(ko == KD - 1),
                        )
                    g_s = silu_into(pgw, Wh)
                    nc.vector.tensor_mul(hW_t[:, ob[0]:ob[1]], g_s[:], puw[:])
                    nc.vector.tensor_mul(
                        hW_t[:, ob[0]:ob[1]], hW_t[:, ob[0]:ob[1]],
                        wmask[:, ob[0]:ob[1]],
                    )
                    nc.vector.tensor_mul(
                        hW_t[:, ib[0]:ib[1]], hA_t[:, ib_lo:ib_lo + Wh],
                        wmask[:, ib[0]:ib[1]],
                    )
                    hA.append(hA_t)
                    hW.append(hW_t)
                for md in range(MD):
                    wd_t = wdp.tile([128, KF, 128], BF16, tag="wd")
                    nc.gpsimd.dma_start(wd_t[:], wdl[e * MD + md])
                    if e == 1:
                        # block first: cols [0, w0) are final right after the
                        # block add, so their output DMA overlaps e=2 compute
                        py = psy.tile([128, HF], F32, tag="y")
                        for kf in range(KF):
                            nc.tensor.matmul(
                                py[:], wd_t[:, kf, :], hA[kf][:],
                                start=(kf == 0), stop=(kf == KF - 1),
                            )
                        nc.vector.tensor_add(
                            yaccs[md][:, off:off + HF],
                            yaccs[md][:, off:off + HF], py[:],
                        )
                        nc.sync.dma_start(yt_r[:, md, 0:w0], yaccs[md][:, 0:w0])
                        pyw = psy.tile([128, W], F32, tag="y", name="pyw")
                        for kf in range(KF):
                            nc.tensor.matmul(
                                pyw[:], wd_t[:, kf, :], hW[kf][:],
                                start=(kf == 0), stop=(kf == KF - 1),
                            )
                        nc.vector.tensor_add(
                            yaccs[md][:, w0:w0 + W],
                            yaccs[md][:, w0:w0 + W], pyw[:],
                        )
                    else:
                        # window first so the final DVE add (and the output
                        # DMA it gates) waits only on the block add
                        pyw = psy.tile([128, W], F32, tag="y", name="pyw")
                        for kf in range(KF):
                            nc.tensor.matmul(
                                pyw[:], wd_t[:, kf, :], hW[kf][:],
                                start=(kf == 0), stop=(kf == KF - 1),
                            )
                        nc.vector.tensor_add(
                            yaccs[md][:, w0:w0 + W],
                            yaccs[md][:, w0:w0 + W], pyw[:],
                        )
                        py = psy.tile([128, HF], F32, tag="y")
                        for kf in range(KF):
                            nc.tensor.matmul(
                                py[:], wd_t[:, kf, :], hA[kf][:],
                                start=(kf == 0), stop=(kf == KF - 1),
                            )
                        nc.vector.tensor_add(
                            yaccs[md][:, off:off + HF],
                            yaccs[md][:, off:off + HF], py[:],
                        )
                        nc.sync.dma_start(yt_r[:, md, w0:T], yaccs[md][:, w0:T])
    return nc


def _pack_weights(W_router, router_bias, Wg, Wu, Wd, Sg, Su, Sd):
    KD, MF, MD, KF = D // 128, F // 128, D // 128, F // 128
    G = np.stack([np.asarray(Sg), np.asarray(Wg)[0], np.asarray(Wg)[1]]).astype(np.float32)
    U = np.stack([np.asarray(Su), np.asarray(Wu)[0], np.asarray(Wu)[1]]).astype(np.float32)
    Dn = np.stack([np.asarray(Sd), np.asarray(Wd)[0], np.asarray(Wd)[1]]).astype(np.float32)
    wgl = np.ascontiguousarray(
        G.reshape(3, KD, 128, MF, 128).transpose(0, 3, 2, 1, 4)
    ).reshape(3 * MF, 128, KD, 128).astype(ml_dtypes.bfloat16)
    wul = np.ascontiguousarray(
        U.reshape(3, KD, 128, MF, 128).transpose(0, 3, 2, 1, 4)
    ).reshape(3 * MF, 128, KD, 128).astype(ml_dtypes.bfloat16)
    wdl = np.ascontiguousarray(
        Dn.reshape(3, KF, 128, MD, 128).transpose(0, 3, 2, 1, 4)
    ).reshape(3 * MD, 128, KF, 128).astype(ml_dtypes.bfloat16)
    wr_h = np.ascontiguousarray(
        np.asarray(W_router, np.float32).reshape(KD, 128, 2).transpose(1, 0, 2)
    )
    rb_h = np.asarray(router_bias, np.float32).reshape(1, 2)
    return wgl, wul, wdl, wr_h, rb_h


def pack_inputs(x, W_router, router_bias, Wg, Wu, Wd, Sg, Su, Sd, T=T, D=D, F=F):
    """Host-side sharding + layout prep for the dense fallback kernel."""
    wgl, wul, wdl, wr_h, rb_h = _pack_weights(
        W_router, router_bias, Wg, Wu, Wd, Sg, Su, Sd
    )
    flat = np.asarray(x, np.float32).reshape(-1, D)
    n_tokens = flat.shape[0]
    assert n_tokens == N_CORES * T
    xt = np.ascontiguousarray(flat.T)  # [D, N]
    xtb_full = xt.astype(ml_dtypes.bfloat16)

    in_maps = []
    for c in range(N_CORES):
        sl = slice(c * T, (c + 1) * T)
        in_maps.append({
            "xt32": np.ascontiguousarray(xt[:, sl]),
            "xtb": np.ascontiguousarray(xtb_full[:, sl]),
            "wr": wr_h,
            "rb": rb_h,
            "wgl": wgl,
            "wul": wul,
            "wdl": wdl,
        })
    return in_maps


def pack_inputs_v4(x, W_router, router_bias, Wg, Wu, Wd, Sg, Su, Sd,
                   T=T, D=D, F=F, W=WINDOW):
    """Host router + global token sort. Returns (in_maps, perms) or None if
    some core's expert split falls outside the static correction window
    (|N0 - N/2| > ~8*(W/2) - 8, a >5-sigma event) -- caller falls back.
    """
    half, Wh = T // 2, W // 2
    w0 = half - Wh
    wgl, wul, wdl, _, _ = _pack_weights(
        W_router, router_bias, Wg, Wu, Wd, Sg, Su, Sd
    )
    flat = np.asarray(x, np.float32).reshape(-1, D)
    n_tokens = flat.shape[0]
    assert n_tokens == N_CORES * T
    logits = flat @ np.asarray(W_router, np.float32)
    logits = logits + np.asarray(router_bias, np.float32)[None, :]
    to_e1 = logits[:, 1] > logits[:, 0]  # ties -> expert 0, like jnp.argmax
    idx0 = np.nonzero(~to_e1)[0]
    idx1 = np.nonzero(to_e1)[0]
    n0 = idx0.size
    base, rem = divmod(n0, N_CORES)
    counts0 = [base + (1 if c < rem else 0) for c in range(N_CORES)]
    if any(not (w0 <= k0 <= half + Wh) for k0 in counts0):
        return None
    in_maps, perms = [], []
    o0 = o1 = 0
    for c in range(N_CORES):
        k0 = counts0[c]
        k1 = T - k0
        perm = np.concatenate([idx0[o0:o0 + k0], idx1[o1:o1 + k1]])
        o0 += k0
        o1 += k1
        xs_c = np.ascontiguousarray(
            flat[perm].T.astype(ml_dtypes.bfloat16)
        )
        # e0-coefficient signed mask over window cols [w0, w0+W):
        # +1 on [half, k0) (e0 tokens computed by block B), -1 on [k0, half)
        # (e1 tokens computed by block A); e1 coefficient is the negation.
        sig = np.zeros((1, W), np.float32)
        if k0 < half:
            sig[0, k0 - w0:half - w0] = -1.0
        elif k0 > half:
            sig[0, half - w0:k0 - w0] = 1.0
        sig_bc = np.ascontiguousarray(
            np.broadcast_to(sig, (128, W)).astype(ml_dtypes.bfloat16)
        )
        sgn_bc = np.ascontiguousarray((-sig_bc).astype(ml_dtypes.bfloat16))
        in_maps.append({
            "xs": xs_c,
            "sig": sig_bc,
            "sgn": sgn_bc,
            "wgl": wgl,
            "wul": wul,
            "wdl": wdl,
        })
        perms.append(perm)
    return in_maps, perms


_CACHE = {}


def _get_compiled(ver="v4"):
    key = f"nc_{ver}"
    if key not in _CACHE:
        nc = bacc.Bacc(
            "TRN2",
            target_bir_lowering=False,
            # axon clients cannot host a BassDebugger; native path can
            debug=not axon_active(),
            num_devices=N_CORES,
        )
        if ver == "v4":
            build_v4(nc, W=WINDOW)
        else:
            build(nc)
        nc.compile()
        _CACHE[key] = nc
    return _CACHE[key]


def _run_v1(np_args, x_shape, _trace=False):
    nc = _get_compiled("v1")
    in_maps = pack_inputs(*np_args)
    res = run_bass_kernel_spmd(
        nc, in_maps, core_ids=list(range(N_CORES)), trace=_trace
    )
    out_t = np.concatenate(
        [res.results[c]["yt"] for c in range(N_CORES)], axis=1
    )
    if _trace:
        _CACHE["last_result"] = res
    return np.ascontiguousarray(out_t.T).reshape(x_shape).astype(np.float32)


def kernel(x, W_router, router_bias, Wg, Wu, Wd, Sg, Su, Sd, _trace=False, **_kw):
    np_args = (x, W_router, router_bias, Wg, Wu, Wd, Sg, Su, Sd)
    x_shape = np.asarray(x).shape
    packed = pack_inputs_v4(*np_args)
    if packed is None:
        # expert split fell outside the static correction window
        # (>5-sigma event for these inputs): run the dense kernel
        return _run_v1(np_args, x_shape, _trace)
    in_maps, perms = packed
    nc = _get_compiled("v4")
    res = run_bass_kernel_spmd(
        nc, in_maps, core_ids=list(range(N_CORES)), trace=_trace
    )
    out = np.empty((N_CORES * T, D), np.float32)
    for c in range(N_CORES):
        # yt columns are in sorted-token order; scatter back
        out[perms[c]] = res.results[c]["yt"].T
    if _trace:
        _CACHE["last_result"] = res
    return out.reshape(x_shape)


# revision 15
# speedup vs baseline: 1.1873x; 1.0092x over previous
import os
import sys

for _p in ("/opt/trn_rl_repo", "/root/.axon_site/_ro/trn_rl_repo"):
    if os.path.isdir(_p) and _p not in sys.path:
        sys.path.insert(0, _p)

import numpy as np
import ml_dtypes

import concourse.bass as bass
import concourse.tile as tile
import concourse.mybir as mybir
from concourse import bacc
from concourse._compat import axon_active
from concourse.bass import ts
from concourse.bass_utils import run_bass_kernel_spmd

N_CORES = 8
D = 1024
F = 2048
T = 1024  # tokens per core (8192 / 8)

BF16 = mybir.dt.bfloat16
F32 = mybir.dt.float32


def build(nc, T=T, D=D, F=F, use_silu=True, psg_b=3, psu_b=3, psy_b=2,
          w_b=4, hb_extra=6, xf_b=3):
    """Dense fallback: per-core MoE FFN with on-device router + masked paths.

    Layout: activations transposed (feature on partitions, tokens on free dim).
    Paths: [shared, expert0, expert1]; expert token masks folded into the
    input (x0 = x*m0, x1 = x - x0) so all three paths sum directly.
    """
    KD = D // 128   # k-tiles over D (gate/up contraction, also out tiles of down)
    MF = F // 128   # m-tiles over F
    MD = D // 128
    KF = F // 128
    TH = T // 512   # 512-token free-dim blocks

    xt32 = nc.dram_tensor("xt32", [D, T], F32, kind="ExternalInput").ap()
    xtb = nc.dram_tensor("xtb", [D, T], BF16, kind="ExternalInput").ap()
    wr = nc.dram_tensor("wr", [128, KD, 2], F32, kind="ExternalInput").ap()
    rb = nc.dram_tensor("rb", [1, 2], F32, kind="ExternalInput").ap()
    wgl = nc.dram_tensor("wgl", [3 * MF, 128, KD, 128], BF16, kind="ExternalInput").ap()
    wul = nc.dram_tensor("wul", [3 * MF, 128, KD, 128], BF16, kind="ExternalInput").ap()
    wdl = nc.dram_tensor("wdl", [3 * MD, 128, KF, 128], BF16, kind="ExternalInput").ap()
    yt = nc.dram_tensor("yt", [D, T], F32, kind="ExternalOutput").ap()

    with tile.TileContext(nc) as tc:
        with (
            tc.tile_pool(name="xres", bufs=1) as xres,
            tc.tile_pool(name="xf", bufs=xf_b) as xf,
            tc.tile_pool(name="small", bufs=1) as small,
            tc.tile_pool(name="wg", bufs=w_b) as wgp,
            tc.tile_pool(name="wu", bufs=w_b) as wup,
            tc.tile_pool(name="wd", bufs=w_b) as wdp,
            tc.tile_pool(name="hb", bufs=KF + hb_extra) as hb,
            tc.tile_pool(name="gs", bufs=3) as gsp,
            tc.tile_pool(name="yac", bufs=1) as yac,
            tc.tile_pool(name="psg", bufs=psg_b, space="PSUM") as psg,
            tc.tile_pool(name="psu", bufs=psu_b, space="PSUM") as psu,
            tc.tile_pool(name="psy", bufs=psy_b, space="PSUM") as psy,
        ):
            # resident transposed input (bf16) + masked variants
            xtb_sb = xres.tile([128, KD, T], BF16, tag="xtb")
            xtb_r = xtb.rearrange("(ko p) t -> p ko t", p=128)
            for ko in range(KD):
                nc.sync.dma_start(xtb_sb[:, ko, :], xtb_r[:, ko, :])
            x0_sb = xres.tile([128, KD, T], BF16, tag="x0")
            x1_sb = xres.tile([128, KD, T], BF16, tag="x1")

            # ---- router (fp32) ----
            wr_sb = small.tile([128, KD, 2], F32, tag="wr")
            nc.sync.dma_start(wr_sb[:], wr)
            wdiff = small.tile([128, KD, 1], F32, tag="wdiff")
            nc.vector.tensor_sub(wdiff[:], wr_sb[:, :, 0:1], wr_sb[:, :, 1:2])
            rb_sb = small.tile([1, 2], F32, tag="rb")
            nc.sync.dma_start(rb_sb[:], rb)
            bdiff = small.tile([1, 1], F32, tag="bdiff")
            nc.vector.tensor_sub(bdiff[:], rb_sb[:, 0:1], rb_sb[:, 1:2])
            ones_sb = small.tile([1, 128], BF16, tag="ones")
            nc.vector.memset(ones_sb[:], 1.0)
            mask_row = small.tile([1, T], BF16, tag="mrow")
            mask_bc = small.tile([128, T], BF16, tag="mbc")

            prs = [
                psg.tile([1, 512], F32, tag="g", name=f"pr{th}") for th in range(TH)
            ]
            for ko in range(KD):
                xf_t = xf.tile([128, T], F32, tag="xf")
                nc.sync.dma_start(xf_t[:], xt32[ko * 128:(ko + 1) * 128, :])
                for th in range(TH):
                    nc.tensor.matmul(
                        prs[th][:], wdiff[:, ko, :], xf_t[:, ts(th, 512)],
                        start=(ko == 0), stop=(ko == KD - 1),
                    )
            # mask0 = ((l0-l1) + (b0-b1)) >= 0, as 1.0/0.0
            for th in range(TH):
                nc.vector.tensor_scalar(
                    mask_row[:, ts(th, 512)], prs[th][:], bdiff[:], 0.0,
                    mybir.AluOpType.add, mybir.AluOpType.is_ge,
                )
            # broadcast mask row across 128 partitions via K=1 matmul with ones
            for th in range(TH):
                pm = psu.tile([128, 512], F32, tag="u")
                nc.tensor.matmul(
                    pm[:], ones_sb[:], mask_row[:, ts(th, 512)], start=True, stop=True
                )
                nc.vector.tensor_copy(mask_bc[:, ts(th, 512)], pm[:])
            for ko in range(KD):
                nc.vector.tensor_mul(x0_sb[:, ko, :], xtb_sb[:, ko, :], mask_bc[:])
                nc.vector.tensor_sub(x1_sb[:, ko, :], xtb_sb[:, ko, :], x0_sb[:, ko, :])

            # ---- 3 SwiGLU paths ----
            yt_r = yt.rearrange("(md p) t -> p md t", p=128)
            yaccs = [
                yac.tile([128, T], F32, tag=f"yacc{md}", name=f"yacc{md}")
                for md in range(MD)
            ]
            xs_by_path = [xtb_sb, x0_sb, x1_sb]
            for p in range(3):
                xp = xs_by_path[p]
                hch = []
                for mf in range(MF):
                    wg_t = wgp.tile([128, KD, 128], BF16, tag="wg")
                    nc.sync.dma_start(wg_t[:], wgl[p * MF + mf])
                    wu_t = wup.tile([128, KD, 128], BF16, tag="wu")
                    nc.sync.dma_start(wu_t[:], wul[p * MF + mf])
                    h_t = hb.tile([128, T], BF16, tag="h")
                    pgs = [
                        psg.tile([128, 512], F32, tag="g", name=f"pg{th}")
                        for th in range(TH)
                    ]
                    pus = [
                        psu.tile([128, 512], F32, tag="u", name=f"pu{th}")
                        for th in range(TH)
                    ]
                    for th in range(TH):
                        for ko in range(KD):
                            nc.tensor.matmul(
                                pgs[th][:], wg_t[:, ko, :], xp[:, ko, ts(th, 512)],
                                start=(ko == 0), stop=(ko == KD - 1),
                            )
                        for ko in range(KD):
                            nc.tensor.matmul(
                                pus[th][:], wu_t[:, ko, :], xp[:, ko, ts(th, 512)],
                                start=(ko == 0), stop=(ko == KD - 1),
                            )
                    for th in range(TH):
                        pg, pu = pgs[th], pus[th]
                        g_s = gsp.tile([128, 512], BF16, tag="gs")
                        if use_silu:
                            nc.scalar.activation(
                                g_s[:], pg[:], mybir.ActivationFunctionType.Silu
                            )
                        else:
                            # CoreSim lacks Silu; g*sigmoid(g) is identical math
                            nc.scalar.activation(
                                g_s[:], pg[:], mybir.ActivationFunctionType.Sigmoid
                            )
                            nc.vector.tensor_mul(g_s[:], g_s[:], pg[:])
                        nc.vector.tensor_mul(h_t[:, ts(th, 512)], g_s[:], pu[:])
                    hch.append(h_t)
                for md in range(MD):
                    wd_t = wdp.tile([128, KF, 128], BF16, tag="wd")
                    nc.sync.dma_start(wd_t[:], wdl[p * MD + md])
                    pys = [
                        psy.tile([128, 512], F32, tag="y", name=f"py{th}")
                        for th in range(TH)
                    ]
                    for kf in range(KF):
                        for th in range(TH):
                            nc.tensor.matmul(
                                pys[th][:], wd_t[:, kf, :], hch[kf][:, ts(th, 512)],
                                start=(kf == 0), stop=(kf == KF - 1),
                            )
                    for th in range(TH):
                        if p == 0:
                            nc.vector.tensor_copy(
                                yaccs[md][:, ts(th, 512)], pys[th][:]
                            )
                        else:
                            nc.vector.tensor_add(
                                yaccs[md][:, ts(th, 512)],
                                yaccs[md][:, ts(th, 512)],
                                pys[th][:],
                            )
                    if p == 2:
                        # final path: this md slice is complete, ship it out
                        nc.sync.dma_start(yt_r[:, md, :], yaccs[md][:])
    return nc


WINDOW = 16


def build_v4(nc, T=T, D=D, F=F, use_silu=True,
             psg_b=2, psu_b=2, psy_b=4, w_b=4, hb_extra=1):
    """Host-routed variant: the host computes the router, globally sorts
    tokens by expert, and hands each core pre-sorted x^T with the expert
    boundary at exactly column T/2 (minority-expert slots zero-filled; the
    |imbalance| displaced tokens are computed host-side in fp32). The device
    runs expert0 on [0, T/2) and expert1 on [T/2, T) unmasked. No on-device
    router / sort metadata / gather / masks; single accumulated output.
    """
    KD = D // 128   # k-tiles over D
    MF = F // 128
    MD = D // 128
    KF = F // 128
    TH = T // 512   # 512-token blocks (shared path free dim)
    half = T // 2
    HF = half       # expert block free dim (= 512, one psum bank)

    xs = nc.dram_tensor("xs", [D, T], BF16, kind="ExternalInput").ap()
    wgl = nc.dram_tensor("wgl", [3 * MF, 128, KD, 128], BF16, kind="ExternalInput").ap()
    wul = nc.dram_tensor("wul", [3 * MF, 128, KD, 128], BF16, kind="ExternalInput").ap()
    wdl = nc.dram_tensor("wdl", [3 * MD, 128, KF, 128], BF16, kind="ExternalInput").ap()
    yt = nc.dram_tensor("yt", [D, T], F32, kind="ExternalOutput").ap()

    AF = mybir.ActivationFunctionType

    with tile.TileContext(nc) as tc:
        with (
            tc.tile_pool(name="xres", bufs=1) as xres,
            tc.tile_pool(name="wg", bufs=w_b) as wgp,
            tc.tile_pool(name="wu", bufs=w_b) as wup,
            tc.tile_pool(name="wd", bufs=w_b) as wdp,
            tc.tile_pool(name="hb", bufs=KF + hb_extra) as hb,
            tc.tile_pool(name="hh", bufs=KF + 1) as hhp,
            tc.tile_pool(name="gs", bufs=3) as gsp,
            tc.tile_pool(name="yac", bufs=1) as yac,
            tc.tile_pool(name="psg", bufs=psg_b, space="PSUM") as psg,
            tc.tile_pool(name="psu", bufs=psu_b, space="PSUM") as psu,
            tc.tile_pool(name="psy", bufs=psy_b, space="PSUM") as psy,
        ):
            # Two DMA rings: weights stream on the Pool/SWDGE ring (gpsimd),
            # activations on the SP/HWDGE ring (sync); their descriptor preps
            # run in parallel so neither stream stalls the other at startup.
            # The first ko slices of wg0/wu0 lead the sync ring so the opening
            # Ldweights fires ~1us earlier; xs strips are split across rings
            # roughly matching each ring's prep rate vs the PE demand order.
            wg_t0 = wgp.tile([128, KD, 128], BF16, tag="wg", name="wg_t0")
            nc.sync.dma_start(wg_t0[:, 0:1, :], wgl[0][:, 0:1, :])
            nc.gpsimd.dma_start(wg_t0[:, 1:KD, :], wgl[0][:, 1:KD, :])
            wu_t0 = wup.tile([128, KD, 128], BF16, tag="wu", name="wu_t0")
            nc.sync.dma_start(wu_t0[:, 0:1, :], wul[0][:, 0:1, :])
            nc.gpsimd.dma_start(wu_t0[:, 1:KD, :], wul[0][:, 1:KD, :])
            xs_sb = xres.tile([128, KD, T], BF16, tag="xs")
            xs_r = xs.rearrange("(ko p) t -> p ko t", p=128)
            sync_strips = {(0, 0), (0, 1), (0, 2), (0, 3), (0, 5),
                           (1, 0), (1, 2), (1, 4), (1, 6)}
            for th in range(TH):
                for ko in range(KD):
                    eng = nc.sync if (th, ko) in sync_strips else nc.gpsimd
                    eng.dma_start(
                        xs_sb[:, ko, ts(th, 512)], xs_r[:, ko, ts(th, 512)]
                    )

            def silu_into(psrc, wdt):
                g_s = gsp.tile([128, wdt], BF16, tag="gs", name="g_s")
                if use_silu:
                    nc.scalar.activation(g_s[:], psrc[:], AF.Silu)
                else:
                    nc.scalar.activation(g_s[:], psrc[:], AF.Sigmoid)
                    nc.vector.tensor_mul(g_s[:], g_s[:], psrc[:])
                return g_s

            yt_r = yt.rearrange("(md p) t -> p md t", p=128)
            yaccs = [
                yac.tile([128, T], F32, tag=f"yacc{md}", name=f"yacc{md}")
                for md in range(MD)
            ]

            # ---- shared path over all (sorted) tokens ----
            # g/u matmuls interleave per-ko so each arriving xs strip feeds
            # two matmuls during the startup trickle
            hch = []
            for mf in range(MF):
                if mf == 0:
                    wg_t, wu_t = wg_t0, wu_t0
                else:
                    wg_t = wgp.tile([128, KD, 128], BF16, tag="wg")
                    nc.gpsimd.dma_start(wg_t[:], wgl[mf])
                    wu_t = wup.tile([128, KD, 128], BF16, tag="wu")
                    nc.gpsimd.dma_start(wu_t[:], wul[mf])
                h_t = hb.tile([128, T], BF16, tag="h")
                for th in range(TH):
                    pg = psg.tile([128, 512], F32, tag="g")
                    pu = psu.tile([128, 512], F32, tag="u")
                    for ko in range(KD):
                        nc.tensor.matmul(
                            pg[:], wg_t[:, ko, :], xs_sb[:, ko, ts(th, 512)],
                            start=(ko == 0), stop=(ko == KD - 1),
                        )
                        nc.tensor.matmul(
                            pu[:], wu_t[:, ko, :], xs_sb[:, ko, ts(th, 512)],
                            start=(ko == 0), stop=(ko == KD - 1),
                        )
                    g_s = silu_into(pg, 512)
                    nc.vector.tensor_mul(h_t[:, ts(th, 512)], g_s[:], pu[:])
                hch.append(h_t)
            for md in range(MD):
                wd_t = wdp.tile([128, KF, 128], BF16, tag="wd")
                nc.gpsimd.dma_start(wd_t[:], wdl[md])
                for th in range(TH):
                    py = psy.tile([128, 512], F32, tag="y")
                    for kf in range(KF):
                        nc.tensor.matmul(
                            py[:], wd_t[:, kf, :], hch[kf][:, ts(th, 512)],
                            start=(kf == 0), stop=(kf == KF - 1),
                        )
                    nc.vector.tensor_copy(yaccs[md][:, ts(th, 512)], py[:])

            # ---- expert blocks (boundary exactly at half; no masks) ----
            for e in (1, 2):
                off = 0 if e == 1 else half
                hA = []
                for mf in range(MF):
                    wg_t = wgp.tile([128, KD, 128], BF16, tag="wg")
                    nc.gpsimd.dma_start(wg_t[:], wgl[e * MF + mf])
                    wu_t = wup.tile([128, KD, 128], BF16, tag="wu")
                    nc.gpsimd.dma_start(wu_t[:], wul[e * MF + mf])
                    hA_t = hhp.tile([128, half], BF16, tag="hh")
                    pg = psg.tile([128, HF], F32, tag="g")
                    pu = psu.tile([128, HF], F32, tag="u")
                    for ko in range(KD):
                        nc.tensor.matmul(
                            pg[:], wg_t[:, ko, :], xs_sb[:, ko, off:off + HF],
                            start=(ko == 0), stop=(ko == KD - 1),
                        )
                        nc.tensor.matmul(
                            pu[:], wu_t[:, ko, :], xs_sb[:, ko, off:off + HF],
                            start=(ko == 0), stop=(ko == KD - 1),
                        )
                    g_s = silu_into(pg, HF)
                    nc.vector.tensor_mul(hA_t[:], g_s[:], pu[:])
                    hA.append(hA_t)
                for md in range(MD):
                    wd_t = wdp.tile([128, KF, 128], BF16, tag="wd")
                    nc.gpsimd.dma_start(wd_t[:], wdl[e * MD + md])
                    py = psy.tile([128, HF], F32, tag="y")
                    for kf in range(KF):
                        nc.tensor.matmul(
                            py[:], wd_t[:, kf, :], hA[kf][:],
                            start=(kf == 0), stop=(kf == KF - 1),
                        )
                    nc.vector.tensor_add(
                        yaccs[md][:, off:off + HF],
                        yaccs[md][:, off:off + HF], py[:],
                    )
                    # this half of the md slice is final: ship it
                    nc.sync.dma_start(
                        yt_r[:, md, off:off + HF], yaccs[md][:, off:off + HF]
                    )
    return nc


def _pack_weights(W_router, router_bias, Wg, Wu, Wd, Sg, Su, Sd):
    KD, MF, MD, KF = D // 128, F // 128, D // 128, F // 128
    G = np.stack([np.asarray(Sg), np.asarray(Wg)[0], np.asarray(Wg)[1]]).astype(np.float32)
    U = np.stack([np.asarray(Su), np.asarray(Wu)[0], np.asarray(Wu)[1]]).astype(np.float32)
    Dn = np.stack([np.asarray(Sd), np.asarray(Wd)[0], np.asarray(Wd)[1]]).astype(np.float32)
    wgl = np.ascontiguousarray(
        G.reshape(3, KD, 128, MF, 128).transpose(0, 3, 2, 1, 4)
    ).reshape(3 * MF, 128, KD, 128).astype(ml_dtypes.bfloat16)
    wul = np.ascontiguousarray(
        U.reshape(3, KD, 128, MF, 128).transpose(0, 3, 2, 1, 4)
    ).reshape(3 * MF, 128, KD, 128).astype(ml_dtypes.bfloat16)
    wdl = np.ascontiguousarray(
        Dn.reshape(3, KF, 128, MD, 128).transpose(0, 3, 2, 1, 4)
    ).reshape(3 * MD, 128, KF, 128).astype(ml_dtypes.bfloat16)
    wr_h = np.ascontiguousarray(
        np.asarray(W_router, np.float32).reshape(KD, 128, 2).transpose(1, 0, 2)
    )
    rb_h = np.asarray(router_bias, np.float32).reshape(1, 2)
    return wgl, wul, wdl, wr_h, rb_h


def pack_inputs(x, W_router, router_bias, Wg, Wu, Wd, Sg, Su, Sd, T=T, D=D, F=F):
    """Host-side sharding + layout prep for the dense fallback kernel."""
    wgl, wul, wdl, wr_h, rb_h = _pack_weights(
        W_router, router_bias, Wg, Wu, Wd, Sg, Su, Sd
    )
    flat = np.asarray(x, np.float32).reshape(-1, D)
    n_tokens = flat.shape[0]
    assert n_tokens == N_CORES * T
    xt = np.ascontiguousarray(flat.T)  # [D, N]
    xtb_full = xt.astype(ml_dtypes.bfloat16)

    in_maps = []
    for c in range(N_CORES):
        sl = slice(c * T, (c + 1) * T)
        in_maps.append({
            "xt32": np.ascontiguousarray(xt[:, sl]),
            "xtb": np.ascontiguousarray(xtb_full[:, sl]),
            "wr": wr_h,
            "rb": rb_h,
            "wgl": wgl,
            "wul": wul,
            "wdl": wdl,
        })
    return in_maps


def _silu32(v):
    return v / (1.0 + np.exp(-v))


def pack_inputs_v4(x, W_router, router_bias, Wg, Wu, Wd, Sg, Su, Sd,
                   T=T, D=D, F=F):
    """Host router + global token sort with the expert boundary pinned to
    exactly T/2 on every core. The majority expert overflows its 4096 slots
    by |d| tokens: those are dropped from the device batch (their slots are
    zero-filled, producing exact zeros through both SwiGLU paths) and
    computed here in fp32. Returns (in_maps, perms, extra) where extra is
    (token_ids, y_host) to overwrite after the device scatter.
    """
    half = T // 2
    wgl, wul, wdl, _, _ = _pack_weights(
        W_router, router_bias, Wg, Wu, Wd, Sg, Su, Sd
    )
    flat = np.asarray(x, np.float32).reshape(-1, D)
    n_tokens = flat.shape[0]
    assert n_tokens == N_CORES * T
    logits = flat @ np.asarray(W_router, np.float32)
    logits = logits + np.asarray(router_bias, np.float32)[None, :]
    to_e1 = logits[:, 1] > logits[:, 0]  # ties -> expert 0, like jnp.argmax
    idx0 = np.nonzero(~to_e1)[0]
    idx1 = np.nonzero(to_e1)[0]
    cap = N_CORES * half
    # overflow tokens of the majority expert: computed host-side in fp32
    drop0 = idx0[cap:]
    drop1 = idx1[cap:]
    idx0 = idx0[:cap]
    idx1 = idx1[:cap]
    in_maps, perms = [], []
    for c in range(N_CORES):
        i0 = idx0[c * half:(c + 1) * half]
        i1 = idx1[c * half:(c + 1) * half]
        k0, k1 = i0.size, i1.size
        xs_c = np.zeros((T, D), np.float32)
        xs_c[0:k0] = flat[i0]
        xs_c[half:half + k1] = flat[i1]
        xs_c = np.ascontiguousarray(xs_c.T.astype(ml_dtypes.bfloat16))
        # slot -> token id; zero-filled slots get -1 (skipped at scatter)
        perm = np.full(T, -1, np.int64)
        perm[0:k0] = i0
        perm[half:half + k1] = i1
        in_maps.append({
            "xs": xs_c,
            "wgl": wgl,
            "wul": wul,
            "wdl": wdl,
        })
        perms.append(perm)
    # fp32 host path for the dropped tokens: shared + their routed expert
    extras = []
    for drop, (eg, eu, ed) in ((drop0, (np.asarray(Wg, np.float32)[0],
                                        np.asarray(Wu, np.float32)[0],
                                        np.asarray(Wd, np.float32)[0])),
                               (drop1, (np.asarray(Wg, np.float32)[1],
                                        np.asarray(Wu, np.float32)[1],
                                        np.asarray(Wd, np.float32)[1]))):
        if drop.size == 0:
            continue
        xv = flat[drop]
        y = (_silu32(xv @ np.asarray(Sg, np.float32))
             * (xv @ np.asarray(Su, np.float32))) @ np.asarray(Sd, np.float32)
        y = y + (_silu32(xv @ eg) * (xv @ eu)) @ ed
        extras.append((drop, y.astype(np.float32)))
    return in_maps, perms, extras


_CACHE = {}


def _get_compiled(ver="v4"):
    key = f"nc_{ver}"
    if key not in _CACHE:
        nc = bacc.Bacc(
            "TRN2",
            target_bir_lowering=False,
            # axon clients cannot host a BassDebugger; native path can
            debug=not axon_active(),
            num_devices=N_CORES,
        )
        if ver == "v4":
            build_v4(nc)
        else:
            build(nc)
        nc.compile()
        _CACHE[key] = nc
    return _CACHE[key]


def _run_v1(np_args, x_shape, _trace=False):
    nc = _get_compiled("v1")
    in_maps = pack_inputs(*np_args)
    res = run_bass_kernel_spmd(
        nc, in_maps, core_ids=list(range(N_CORES)), trace=_trace
    )
    out_t = np.concatenate(
        [res.results[c]["yt"] for c in range(N_CORES)], axis=1
    )
    if _trace:
        _CACHE["last_result"] = res
    return np.ascontiguousarray(out_t.T).reshape(x_shape).astype(np.float32)


def kernel(x, W_router, router_bias, Wg, Wu, Wd, Sg, Su, Sd, _trace=False, **_kw):
    np_args = (x, W_router, router_bias, Wg, Wu, Wd, Sg, Su, Sd)
    x_shape = np.asarray(x).shape
    in_maps, perms, extras = pack_inputs_v4(*np_args)
    nc = _get_compiled("v4")
    res = run_bass_kernel_spmd(
        nc, in_maps, core_ids=list(range(N_CORES)), trace=_trace
    )
    out = np.empty((N_CORES * T, D), np.float32)
    for c in range(N_CORES):
        # yt columns are in sorted-slot order; scatter real slots back
        perm = perms[c]
        valid = perm >= 0
        out[perm[valid]] = res.results[c]["yt"].T[valid]
    for ids, y in extras:
        out[ids] = y
    if _trace:
        _CACHE["last_result"] = res
    return out.reshape(x_shape)


# revision 18
# speedup vs baseline: 1.1936x; 1.0053x over previous
import os
import sys

for _p in ("/opt/trn_rl_repo", "/root/.axon_site/_ro/trn_rl_repo"):
    if os.path.isdir(_p) and _p not in sys.path:
        sys.path.insert(0, _p)

import numpy as np
import ml_dtypes

import concourse.bass as bass
import concourse.tile as tile
import concourse.mybir as mybir
from concourse import bacc
from concourse._compat import axon_active
from concourse.bass import ts
from concourse.bass_utils import run_bass_kernel_spmd

N_CORES = 8
D = 1024
F = 2048
T = 1024  # tokens per core (8192 / 8)

BF16 = mybir.dt.bfloat16
F32 = mybir.dt.float32


def build(nc, T=T, D=D, F=F, use_silu=True, psg_b=3, psu_b=3, psy_b=2,
          w_b=4, hb_extra=6, xf_b=3):
    """Dense fallback: per-core MoE FFN with on-device router + masked paths.

    Layout: activations transposed (feature on partitions, tokens on free dim).
    Paths: [shared, expert0, expert1]; expert token masks folded into the
    input (x0 = x*m0, x1 = x - x0) so all three paths sum directly.
    """
    KD = D // 128   # k-tiles over D (gate/up contraction, also out tiles of down)
    MF = F // 128   # m-tiles over F
    MD = D // 128
    KF = F // 128
    TH = T // 512   # 512-token free-dim blocks

    xt32 = nc.dram_tensor("xt32", [D, T], F32, kind="ExternalInput").ap()
    xtb = nc.dram_tensor("xtb", [D, T], BF16, kind="ExternalInput").ap()
    wr = nc.dram_tensor("wr", [128, KD, 2], F32, kind="ExternalInput").ap()
    rb = nc.dram_tensor("rb", [1, 2], F32, kind="ExternalInput").ap()
    wgl = nc.dram_tensor("wgl", [3 * MF, 128, KD, 128], BF16, kind="ExternalInput").ap()
    wul = nc.dram_tensor("wul", [3 * MF, 128, KD, 128], BF16, kind="ExternalInput").ap()
    wdl = nc.dram_tensor("wdl", [3 * MD, 128, KF, 128], BF16, kind="ExternalInput").ap()
    yt = nc.dram_tensor("yt", [D, T], F32, kind="ExternalOutput").ap()

    with tile.TileContext(nc) as tc:
        with (
            tc.tile_pool(name="xres", bufs=1) as xres,
            tc.tile_pool(name="xf", bufs=xf_b) as xf,
            tc.tile_pool(name="small", bufs=1) as small,
            tc.tile_pool(name="wg", bufs=w_b) as wgp,
            tc.tile_pool(name="wu", bufs=w_b) as wup,
            tc.tile_pool(name="wd", bufs=w_b) as wdp,
            tc.tile_pool(name="hb", bufs=KF + hb_extra) as hb,
            tc.tile_pool(name="gs", bufs=3) as gsp,
            tc.tile_pool(name="yac", bufs=1) as yac,
            tc.tile_pool(name="psg", bufs=psg_b, space="PSUM") as psg,
            tc.tile_pool(name="psu", bufs=psu_b, space="PSUM") as psu,
            tc.tile_pool(name="psy", bufs=psy_b, space="PSUM") as psy,
        ):
            # resident transposed input (bf16) + masked variants
            xtb_sb = xres.tile([128, KD, T], BF16, tag="xtb")
            xtb_r = xtb.rearrange("(ko p) t -> p ko t", p=128)
            for ko in range(KD):
                nc.sync.dma_start(xtb_sb[:, ko, :], xtb_r[:, ko, :])
            x0_sb = xres.tile([128, KD, T], BF16, tag="x0")
            x1_sb = xres.tile([128, KD, T], BF16, tag="x1")

            # ---- router (fp32) ----
            wr_sb = small.tile([128, KD, 2], F32, tag="wr")
            nc.sync.dma_start(wr_sb[:], wr)
            wdiff = small.tile([128, KD, 1], F32, tag="wdiff")
            nc.vector.tensor_sub(wdiff[:], wr_sb[:, :, 0:1], wr_sb[:, :, 1:2])
            rb_sb = small.tile([1, 2], F32, tag="rb")
            nc.sync.dma_start(rb_sb[:], rb)
            bdiff = small.tile([1, 1], F32, tag="bdiff")
            nc.vector.tensor_sub(bdiff[:], rb_sb[:, 0:1], rb_sb[:, 1:2])
            ones_sb = small.tile([1, 128], BF16, tag="ones")
            nc.vector.memset(ones_sb[:], 1.0)
            mask_row = small.tile([1, T], BF16, tag="mrow")
            mask_bc = small.tile([128, T], BF16, tag="mbc")

            prs = [
                psg.tile([1, 512], F32, tag="g", name=f"pr{th}") for th in range(TH)
            ]
            for ko in range(KD):
                xf_t = xf.tile([128, T], F32, tag="xf")
                nc.sync.dma_start(xf_t[:], xt32[ko * 128:(ko + 1) * 128, :])
                for th in range(TH):
                    nc.tensor.matmul(
                        prs[th][:], wdiff[:, ko, :], xf_t[:, ts(th, 512)],
                        start=(ko == 0), stop=(ko == KD - 1),
                    )
            # mask0 = ((l0-l1) + (b0-b1)) >= 0, as 1.0/0.0
            for th in range(TH):
                nc.vector.tensor_scalar(
                    mask_row[:, ts(th, 512)], prs[th][:], bdiff[:], 0.0,
                    mybir.AluOpType.add, mybir.AluOpType.is_ge,
                )
            # broadcast mask row across 128 partitions via K=1 matmul with ones
            for th in range(TH):
                pm = psu.tile([128, 512], F32, tag="u")
                nc.tensor.matmul(
                    pm[:], ones_sb[:], mask_row[:, ts(th, 512)], start=True, stop=True
                )
                nc.vector.tensor_copy(mask_bc[:, ts(th, 512)], pm[:])
            for ko in range(KD):
                nc.vector.tensor_mul(x0_sb[:, ko, :], xtb_sb[:, ko, :], mask_bc[:])
                nc.vector.tensor_sub(x1_sb[:, ko, :], xtb_sb[:, ko, :], x0_sb[:, ko, :])

            # ---- 3 SwiGLU paths ----
            yt_r = yt.rearrange("(md p) t -> p md t", p=128)
            yaccs = [
                yac.tile([128, T], F32, tag=f"yacc{md}", name=f"yacc{md}")
                for md in range(MD)
            ]
            xs_by_path = [xtb_sb, x0_sb, x1_sb]
            for p in range(3):
                xp = xs_by_path[p]
                hch = []
                for mf in range(MF):
                    wg_t = wgp.tile([128, KD, 128], BF16, tag="wg")
                    nc.sync.dma_start(wg_t[:], wgl[p * MF + mf])
                    wu_t = wup.tile([128, KD, 128], BF16, tag="wu")
                    nc.sync.dma_start(wu_t[:], wul[p * MF + mf])
                    h_t = hb.tile([128, T], BF16, tag="h")
                    pgs = [
                        psg.tile([128, 512], F32, tag="g", name=f"pg{th}")
                        for th in range(TH)
                    ]
                    pus = [
                        psu.tile([128, 512], F32, tag="u", name=f"pu{th}")
                        for th in range(TH)
                    ]
                    for th in range(TH):
                        for ko in range(KD):
                            nc.tensor.matmul(
                                pgs[th][:], wg_t[:, ko, :], xp[:, ko, ts(th, 512)],
                                start=(ko == 0), stop=(ko == KD - 1),
                            )
                        for ko in range(KD):
                            nc.tensor.matmul(
                                pus[th][:], wu_t[:, ko, :], xp[:, ko, ts(th, 512)],
                                start=(ko == 0), stop=(ko == KD - 1),
                            )
                    for th in range(TH):
                        pg, pu = pgs[th], pus[th]
                        g_s = gsp.tile([128, 512], BF16, tag="gs")
                        if use_silu:
                            nc.scalar.activation(
                                g_s[:], pg[:], mybir.ActivationFunctionType.Silu
                            )
                        else:
                            # CoreSim lacks Silu; g*sigmoid(g) is identical math
                            nc.scalar.activation(
                                g_s[:], pg[:], mybir.ActivationFunctionType.Sigmoid
                            )
                            nc.vector.tensor_mul(g_s[:], g_s[:], pg[:])
                        nc.vector.tensor_mul(h_t[:, ts(th, 512)], g_s[:], pu[:])
                    hch.append(h_t)
                for md in range(MD):
                    wd_t = wdp.tile([128, KF, 128], BF16, tag="wd")
                    nc.sync.dma_start(wd_t[:], wdl[p * MD + md])
                    pys = [
                        psy.tile([128, 512], F32, tag="y", name=f"py{th}")
                        for th in range(TH)
                    ]
                    for kf in range(KF):
                        for th in range(TH):
                            nc.tensor.matmul(
                                pys[th][:], wd_t[:, kf, :], hch[kf][:, ts(th, 512)],
                                start=(kf == 0), stop=(kf == KF - 1),
                            )
                    for th in range(TH):
                        if p == 0:
                            nc.vector.tensor_copy(
                                yaccs[md][:, ts(th, 512)], pys[th][:]
                            )
                        else:
                            nc.vector.tensor_add(
                                yaccs[md][:, ts(th, 512)],
                                yaccs[md][:, ts(th, 512)],
                                pys[th][:],
                            )
                    if p == 2:
                        # final path: this md slice is complete, ship it out
                        nc.sync.dma_start(yt_r[:, md, :], yaccs[md][:])
    return nc


WINDOW = 16


def build_v4(nc, T=T, D=D, F=F, use_silu=True,
             psg_b=2, psu_b=2, psy_b=4, w_b=4, hb_extra=1):
    """Host-routed variant: the host computes the router, globally sorts
    tokens by expert, and hands each core pre-sorted x^T with the expert
    boundary at exactly column T/2 (minority-expert slots zero-filled; the
    |imbalance| displaced tokens are computed host-side in fp32). The device
    runs expert0 on [0, T/2) and expert1 on [T/2, T) unmasked. No on-device
    router / sort metadata / gather / masks; single accumulated output.
    """
    KD = D // 128   # k-tiles over D
    MF = F // 128
    MD = D // 128
    KF = F // 128
    TH = T // 512   # 512-token blocks (shared path free dim)
    half = T // 2
    HF = half       # expert block free dim (= 512, one psum bank)

    xs = nc.dram_tensor("xs", [D, T], BF16, kind="ExternalInput").ap()
    wgl = nc.dram_tensor("wgl", [3 * MF, 128, KD, 128], BF16, kind="ExternalInput").ap()
    wul = nc.dram_tensor("wul", [3 * MF, 128, KD, 128], BF16, kind="ExternalInput").ap()
    wdl = nc.dram_tensor("wdl", [3 * MD, 128, KF, 128], BF16, kind="ExternalInput").ap()
    yt = nc.dram_tensor("yt", [D, T], F32, kind="ExternalOutput").ap()

    AF = mybir.ActivationFunctionType

    with tile.TileContext(nc) as tc:
        with (
            tc.tile_pool(name="xres", bufs=1) as xres,
            tc.tile_pool(name="wg", bufs=w_b) as wgp,
            tc.tile_pool(name="wu", bufs=w_b) as wup,
            tc.tile_pool(name="wd", bufs=w_b) as wdp,
            tc.tile_pool(name="hb", bufs=KF + hb_extra) as hb,
            tc.tile_pool(name="hh", bufs=KF + 1) as hhp,
            tc.tile_pool(name="gs", bufs=3) as gsp,
            tc.tile_pool(name="yac", bufs=1) as yac,
            tc.tile_pool(name="psg", bufs=psg_b, space="PSUM") as psg,
            tc.tile_pool(name="psu", bufs=psu_b, space="PSUM") as psu,
            tc.tile_pool(name="psy", bufs=psy_b, space="PSUM") as psy,
        ):
            # Two DMA rings: weights stream on the Pool/SWDGE ring (gpsimd),
            # activations on the SP/HWDGE ring (sync); their descriptor preps
            # run in parallel so neither stream stalls the other at startup.
            # The first ko slices of wg0/wu0 lead the sync ring so the opening
            # Ldweights fires ~1us earlier; xs strips are split across rings
            # roughly matching each ring's prep rate vs the PE demand order.
            wg_t0 = wgp.tile([128, KD, 128], BF16, tag="wg", name="wg_t0")
            nc.sync.dma_start(wg_t0[:, 0:1, :], wgl[0][:, 0:1, :])
            nc.gpsimd.dma_start(wg_t0[:, 1:KD, :], wgl[0][:, 1:KD, :])
            wu_t0 = wup.tile([128, KD, 128], BF16, tag="wu", name="wu_t0")
            nc.gpsimd.dma_start(wu_t0[:], wul[0])
            xs_sb = xres.tile([128, KD, T], BF16, tag="xs")
            xs_r = xs.rearrange("(ko p) t -> p ko t", p=128)
            # th0: mostly sync (fast ring) in consumption order; th1 all on
            # sync so the pool ring reaches the mf=1 weights early
            pool_strips = {(0, 4), (0, 6), (0, 7)}
            for th in range(TH):
                for ko in range(KD):
                    eng = nc.gpsimd if (th, ko) in pool_strips else nc.sync
                    eng.dma_start(
                        xs_sb[:, ko, ts(th, 512)], xs_r[:, ko, ts(th, 512)]
                    )

            def silu_into(psrc, wdt):
                g_s = gsp.tile([128, wdt], BF16, tag="gs", name="g_s")
                if use_silu:
                    nc.scalar.activation(g_s[:], psrc[:], AF.Silu)
                else:
                    nc.scalar.activation(g_s[:], psrc[:], AF.Sigmoid)
                    nc.vector.tensor_mul(g_s[:], g_s[:], psrc[:])
                return g_s

            yt_r = yt.rearrange("(md p) t -> p md t", p=128)
            yaccs = [
                yac.tile([128, T], F32, tag=f"yacc{md}", name=f"yacc{md}")
                for md in range(MD)
            ]

            # ---- shared path over all (sorted) tokens ----
            # g/u matmuls interleave per-ko so each arriving xs strip feeds
            # two matmuls during the startup trickle
            hch = []
            for mf in range(MF):
                if mf == 0:
                    wg_t, wu_t = wg_t0, wu_t0
                else:
                    wg_t = wgp.tile([128, KD, 128], BF16, tag="wg")
                    nc.gpsimd.dma_start(wg_t[:], wgl[mf])
                    wu_t = wup.tile([128, KD, 128], BF16, tag="wu")
                    nc.gpsimd.dma_start(wu_t[:], wul[mf])
                h_t = hb.tile([128, T], BF16, tag="h")
                for th in range(TH):
                    pg = psg.tile([128, 512], F32, tag="g")
                    pu = psu.tile([128, 512], F32, tag="u")
                    for ko in range(KD):
                        nc.tensor.matmul(
                            pg[:], wg_t[:, ko, :], xs_sb[:, ko, ts(th, 512)],
                            start=(ko == 0), stop=(ko == KD - 1),
                        )
                        nc.tensor.matmul(
                            pu[:], wu_t[:, ko, :], xs_sb[:, ko, ts(th, 512)],
                            start=(ko == 0), stop=(ko == KD - 1),
                        )
                    g_s = silu_into(pg, 512)
                    nc.vector.tensor_mul(h_t[:, ts(th, 512)], g_s[:], pu[:])
                hch.append(h_t)
            for md in range(MD):
                wd_t = wdp.tile([128, KF, 128], BF16, tag="wd")
                nc.gpsimd.dma_start(wd_t[:], wdl[md])
                for th in range(TH):
                    py = psy.tile([128, 512], F32, tag="y")
                    for kf in range(KF):
                        nc.tensor.matmul(
                            py[:], wd_t[:, kf, :], hch[kf][:, ts(th, 512)],
                            start=(kf == 0), stop=(kf == KF - 1),
                        )
                    nc.vector.tensor_copy(yaccs[md][:, ts(th, 512)], py[:])

            # ---- expert blocks (boundary exactly at half; no masks) ----
            for e in (1, 2):
                off = 0 if e == 1 else half
                hA = []
                for mf in range(MF):
                    wg_t = wgp.tile([128, KD, 128], BF16, tag="wg")
                    nc.gpsimd.dma_start(wg_t[:], wgl[e * MF + mf])
                    wu_t = wup.tile([128, KD, 128], BF16, tag="wu")
                    nc.gpsimd.dma_start(wu_t[:], wul[e * MF + mf])
                    hA_t = hhp.tile([128, half], BF16, tag="hh")
                    pg = psg.tile([128, HF], F32, tag="g")
                    pu = psu.tile([128, HF], F32, tag="u")
                    for ko in range(KD):
                        nc.tensor.matmul(
                            pg[:], wg_t[:, ko, :], xs_sb[:, ko, off:off + HF],
                            start=(ko == 0), stop=(ko == KD - 1),
                        )
                        nc.tensor.matmul(
                            pu[:], wu_t[:, ko, :], xs_sb[:, ko, off:off + HF],
                            start=(ko == 0), stop=(ko == KD - 1),
                        )
                    g_s = silu_into(pg, HF)
                    nc.vector.tensor_mul(hA_t[:], g_s[:], pu[:])
                    hA.append(hA_t)
                for md in range(MD):
                    wd_t = wdp.tile([128, KF, 128], BF16, tag="wd")
                    nc.gpsimd.dma_start(wd_t[:], wdl[e * MD + md])
                    # the very last block runs as two half-width chunks so the
                    # closing add+DMA chain covers 256 cols instead of 512
                    chunks = 2 if (e == 2 and md == MD - 1) else 1
                    cw = HF // chunks
                    for ch in range(chunks):
                        o2 = off + ch * cw
                        py = psy.tile([128, cw], F32, tag="y")
                        for kf in range(KF):
                            nc.tensor.matmul(
                                py[:], wd_t[:, kf, :],
                                hA[kf][:, ch * cw:(ch + 1) * cw],
                                start=(kf == 0), stop=(kf == KF - 1),
                            )
                        nc.vector.tensor_add(
                            yaccs[md][:, o2:o2 + cw],
                            yaccs[md][:, o2:o2 + cw], py[:],
                        )
                        # this slice of the md row is final: ship it
                        nc.sync.dma_start(
                            yt_r[:, md, o2:o2 + cw], yaccs[md][:, o2:o2 + cw]
                        )
    return nc


def _pack_weights(W_router, router_bias, Wg, Wu, Wd, Sg, Su, Sd):
    KD, MF, MD, KF = D // 128, F // 128, D // 128, F // 128
    G = np.stack([np.asarray(Sg), np.asarray(Wg)[0], np.asarray(Wg)[1]]).astype(np.float32)
    U = np.stack([np.asarray(Su), np.asarray(Wu)[0], np.asarray(Wu)[1]]).astype(np.float32)
    Dn = np.stack([np.asarray(Sd), np.asarray(Wd)[0], np.asarray(Wd)[1]]).astype(np.float32)
    wgl = np.ascontiguousarray(
        G.reshape(3, KD, 128, MF, 128).transpose(0, 3, 2, 1, 4)
    ).reshape(3 * MF, 128, KD, 128).astype(ml_dtypes.bfloat16)
    wul = np.ascontiguousarray(
        U.reshape(3, KD, 128, MF, 128).transpose(0, 3, 2, 1, 4)
    ).reshape(3 * MF, 128, KD, 128).astype(ml_dtypes.bfloat16)
    wdl = np.ascontiguousarray(
        Dn.reshape(3, KF, 128, MD, 128).transpose(0, 3, 2, 1, 4)
    ).reshape(3 * MD, 128, KF, 128).astype(ml_dtypes.bfloat16)
    wr_h = np.ascontiguousarray(
        np.asarray(W_router, np.float32).reshape(KD, 128, 2).transpose(1, 0, 2)
    )
    rb_h = np.asarray(router_bias, np.float32).reshape(1, 2)
    return wgl, wul, wdl, wr_h, rb_h


def pack_inputs(x, W_router, router_bias, Wg, Wu, Wd, Sg, Su, Sd, T=T, D=D, F=F):
    """Host-side sharding + layout prep for the dense fallback kernel."""
    wgl, wul, wdl, wr_h, rb_h = _pack_weights(
        W_router, router_bias, Wg, Wu, Wd, Sg, Su, Sd
    )
    flat = np.asarray(x, np.float32).reshape(-1, D)
    n_tokens = flat.shape[0]
    assert n_tokens == N_CORES * T
    xt = np.ascontiguousarray(flat.T)  # [D, N]
    xtb_full = xt.astype(ml_dtypes.bfloat16)

    in_maps = []
    for c in range(N_CORES):
        sl = slice(c * T, (c + 1) * T)
        in_maps.append({
            "xt32": np.ascontiguousarray(xt[:, sl]),
            "xtb": np.ascontiguousarray(xtb_full[:, sl]),
            "wr": wr_h,
            "rb": rb_h,
            "wgl": wgl,
            "wul": wul,
            "wdl": wdl,
        })
    return in_maps


def _silu32(v):
    return v / (1.0 + np.exp(-v))


def pack_inputs_v4(x, W_router, router_bias, Wg, Wu, Wd, Sg, Su, Sd,
                   T=T, D=D, F=F):
    """Host router + global token sort with the expert boundary pinned to
    exactly T/2 on every core. The majority expert overflows its 4096 slots
    by |d| tokens: those are dropped from the device batch (their slots are
    zero-filled, producing exact zeros through both SwiGLU paths) and
    computed here in fp32. Returns (in_maps, perms, extra) where extra is
    (token_ids, y_host) to overwrite after the device scatter.
    """
    half = T // 2
    wgl, wul, wdl, _, _ = _pack_weights(
        W_router, router_bias, Wg, Wu, Wd, Sg, Su, Sd
    )
    flat = np.asarray(x, np.float32).reshape(-1, D)
    n_tokens = flat.shape[0]
    assert n_tokens == N_CORES * T
    logits = flat @ np.asarray(W_router, np.float32)
    logits = logits + np.asarray(router_bias, np.float32)[None, :]
    to_e1 = logits[:, 1] > logits[:, 0]  # ties -> expert 0, like jnp.argmax
    idx0 = np.nonzero(~to_e1)[0]
    idx1 = np.nonzero(to_e1)[0]
    cap = N_CORES * half
    # overflow tokens of the majority expert: computed host-side in fp32
    drop0 = idx0[cap:]
    drop1 = idx1[cap:]
    idx0 = idx0[:cap]
    idx1 = idx1[:cap]
    in_maps, perms = [], []
    for c in range(N_CORES):
        i0 = idx0[c * half:(c + 1) * half]
        i1 = idx1[c * half:(c + 1) * half]
        k0, k1 = i0.size, i1.size
        xs_c = np.zeros((T, D), np.float32)
        xs_c[0:k0] = flat[i0]
        xs_c[half:half + k1] = flat[i1]
        xs_c = np.ascontiguousarray(xs_c.T.astype(ml_dtypes.bfloat16))
        # slot -> token id; zero-filled slots get -1 (skipped at scatter)
        perm = np.full(T, -1, np.int64)
        perm[0:k0] = i0
        perm[half:half + k1] = i1
        in_maps.append({
            "xs": xs_c,
            "wgl": wgl,
            "wul": wul,
            "wdl": wdl,
        })
        perms.append(perm)
    # fp32 host path for the dropped tokens: shared + their routed expert
    extras = []
    for drop, (eg, eu, ed) in ((drop0, (np.asarray(Wg, np.float32)[0],
                                        np.asarray(Wu, np.float32)[0],
                                        np.asarray(Wd, np.float32)[0])),
                               (drop1, (np.asarray(Wg, np.float32)[1],
                                        np.asarray(Wu, np.float32)[1],
                                        np.asarray(Wd, np.float32)[1]))):
        if drop.size == 0:
            continue
        xv = flat[drop]
        y = (_silu32(xv @ np.asarray(Sg, np.float32))
             * (xv @ np.asarray(Su, np.float32))) @ np.asarray(Sd, np.float32)
        y = y + (_silu32(xv @ eg) * (xv @ eu)) @ ed
        extras.append((drop, y.astype(np.float32)))
    return in_maps, perms, extras


_CACHE = {}


def _get_compiled(ver="v4"):
    key = f"nc_{ver}"
    if key not in _CACHE:
        nc = bacc.Bacc(
            "TRN2",
            target_bir_lowering=False,
            # axon clients cannot host a BassDebugger; native path can
            debug=not axon_active(),
            num_devices=N_CORES,
        )
        if ver == "v4":
            build_v4(nc)
        else:
            build(nc)
        nc.compile()
        _CACHE[key] = nc
    return _CACHE[key]


def _run_v1(np_args, x_shape, _trace=False):
    nc = _get_compiled("v1")
    in_maps = pack_inputs(*np_args)
    res = run_bass_kernel_spmd(
        nc, in_maps, core_ids=list(range(N_CORES)), trace=_trace
    )
    out_t = np.concatenate(
        [res.results[c]["yt"] for c in range(N_CORES)], axis=1
    )
    if _trace:
        _CACHE["last_result"] = res
    return np.ascontiguousarray(out_t.T).reshape(x_shape).astype(np.float32)


def kernel(x, W_router, router_bias, Wg, Wu, Wd, Sg, Su, Sd, _trace=False, **_kw):
    np_args = (x, W_router, router_bias, Wg, Wu, Wd, Sg, Su, Sd)
    x_shape = np.asarray(x).shape
    in_maps, perms, extras = pack_inputs_v4(*np_args)
    nc = _get_compiled("v4")
    res = run_bass_kernel_spmd(
        nc, in_maps, core_ids=list(range(N_CORES)), trace=_trace
    )
    out = np.empty((N_CORES * T, D), np.float32)
    for c in range(N_CORES):
        # yt columns are in sorted-slot order; scatter real slots back
        perm = perms[c]
        valid = perm >= 0
        out[perm[valid]] = res.results[c]["yt"].T[valid]
    for ids, y in extras:
        out[ids] = y
    if _trace:
        _CACHE["last_result"] = res
    return out.reshape(x_shape)
